# revision 31
# baseline (speedup 1.0000x reference)
"""MiniMax-M2 sparse MoE block on 8 Trainium2 NeuronCores (expert-parallel).

v2: load-balanced adaptive schedule + split-precision router.

T=4096 tokens, H=1536, I=768, E=64 experts, top-8 sigmoid routing.

Key ideas vs v1 (8 static experts/core x 7 capacity tiles = 56 row-tiles):
  * PE matmul cost scales with the output free-dim only, so P3 cost =
    (#row-tiles) x ~11.8us.  The per-core minimum is
    ceil(sum_e ceil(load_e/128) / 8) ~= 38 tiles.  A host-side scheduler
    (the router is recomputed on host in fp32 purely to pick the schedule;
    all math still runs on device) bin-packs experts into 8 cores with an
    identical per-slot tile profile, splitting experts across cores when
    needed.
  * Split-precision router: x = xh + xl (bf16 each), gw = gh + gl;
    logits = xh@gh + xl@gh + xh@gl accumulated in fp32 PSUM.  Zero top-8
    flips vs fp32 (logit err ~1.6e-5) at bf16 matmul speed.  The
    transposed x operands are host-prepared, so P1 needs no PE transposes.
  * Dispatch (P2) generalized to per-slot (expert, start, ntiles) pieces;
    all per-slot binding constants are input data, only the slot
    tile-count profile is program structure (identical across cores ->
    SPMD-safe).  local_scatter uses 8-lane channels (16 slots max in one
    128-channel page); two matmuls per slot re-wrap the lists into the
    16-lane replicated layout the SWDGE gathers expect.
  * pout accumulates in bf16 (halves scatter-add traffic); host sums the
    8 partials in fp32.
"""

import math

import numpy as np
import ml_dtypes

import concourse.bass as bass
import concourse.mybir as mybir
import concourse.tile as tile
from concourse import bacc, library_config
from concourse import bass_utils
from concourse.bass import _add_dep_helper

BF16 = ml_dtypes.bfloat16

T = 4096
H = 1536
II = 768
E = 64
K = 8
NCORES = 8
PAD = 16          # safety margin on host-predicted expert loads
TP = T + 16       # padded token rows; rows T.. are the zero sentinel
AF = mybir.ActivationFunctionType
ALU = mybir.AluOpType
F32 = mybir.dt.float32
F32R = mybir.dt.float32r
BF = mybir.dt.bfloat16
I16 = mybir.dt.int16
HC = H // 128      # 12
IC = II // 128     # 6
NCHUNK = T // 128  # 32
TH = T // 2
GP = 16            # gatS width / max slots
NSTRIP = 8
SW = T // NSTRIP   # 512


# --------------------------------------------------------------------------
# host-side routing + scheduling
# --------------------------------------------------------------------------

def _host_loads(x, gw, rb):
    """fp32 router on host; per-expert token counts (schedule only)."""
    logits = x.astype(np.float32) @ gw.astype(np.float32).T
    scores = 1.0 / (1.0 + np.exp(-logits))
    sel = scores + rb.astype(np.float32)[None, :]
    kth = np.partition(sel, E - K, axis=1)[:, E - K][:, None]
    return (sel >= kth).sum(axis=0).astype(int)


def _schedule(loads):
    """Bin-pack experts into 8 cores x G slots.

    Returns (profile, bins): profile = per-slot tile counts (descending,
    shared across cores); bins[c] = list of (expert, start_tile, ntiles)
    sorted descending by ntiles, padded with (-1, 0, 0) to len(profile).
    No expert appears twice in one bin.
    """
    import random
    tiles = [max(1, math.ceil((l + PAD) / 128)) for l in loads]
    TT = sum(tiles)
    Q = math.ceil(TT / NCORES)

    def rankmax(bins):
        g = max(len(b) for b in bins)
        return sum(max((b[r][2] if r < len(b) else 0) for b in bins)
                   for r in range(g))

    def valid(b):
        return len({p[0] for p in b}) == len(b)

    def lpt():
        order = sorted(range(E), key=lambda e: -tiles[e])
        bins = [[] for _ in range(NCORES)]
        sums = [0] * NCORES
        for e in order:
            c = min(range(NCORES), key=lambda i: (sums[i], len(bins[i])))
            bins[c].append((e, 0, tiles[e]))
            sums[c] += tiles[e]
        for b in bins:
            b.sort(key=lambda p: -p[2])
        return bins

    def split_fill():
        order = sorted(range(E), key=lambda e: -tiles[e])
        bins = [[] for _ in range(NCORES)]
        c, rem = 0, Q
        for e in order:
            t, st = tiles[e], 0
            while t > 0:
                if rem == 0 and c < NCORES - 1:
                    c += 1
                    rem = Q
                take = min(t, rem)
                bins[c].append((e, st, take))
                st += take
                t -= take
                rem -= take
        for b in bins:
            b.sort(key=lambda p: -p[2])
        return bins

    rng = random.Random(0)

    def cost_of(bins):
        return rankmax(bins) * 100 + max(len(b) for b in bins)

    def local_search(base, iters=20000):
        bins = [list(b) for b in base]
        best = cost_of(bins)
        for _ in range(iters):
            c1, c2 = rng.randrange(NCORES), rng.randrange(NCORES)
            if c1 == c2 or not bins[c1]:
                continue
            i1 = rng.randrange(len(bins[c1]))
            saved = (list(bins[c1]), list(bins[c2]))
            r = rng.random()
            if r < 0.4 and bins[c2]:
                i2 = rng.randrange(len(bins[c2]))
                bins[c1][i1], bins[c2][i2] = bins[c2][i2], bins[c1][i1]
            elif r < 0.7:
                bins[c2].append(bins[c1].pop(i1))
            else:
                e, st, nt = bins[c1][i1]
                if nt < 2:
                    continue
                k = rng.randrange(1, nt)
                bins[c1][i1] = (e, st, nt - k)
                bins[c2].append((e, st + nt - k, k))
            if valid(bins[c1]) and valid(bins[c2]) and \
                    max(len(bins[c1]), len(bins[c2])) <= GP:
                for b in (bins[c1], bins[c2]):
                    b.sort(key=lambda p: -p[2])
                cost = cost_of(bins)
                if cost <= best:
                    best = cost
                    continue
            bins[c1], bins[c2] = saved
        return bins

    def profile_construct():
        """Enumerate slot profiles (desc partitions of S, parts<=7) in
        (S, nslots) order; rank-deal items into 8 bins with splitting;
        return bins for the first feasible profile."""
        maxpart = min(7, max(tiles))

        def partitions(S, maxp, maxlen):
            if S == 0:
                yield []
                return
            if maxlen == 0:
                return
            for p in range(min(maxp, S), 0, -1):
                for rest in partitions(S - p, p, maxlen - 1):
                    yield [p] + rest

        import bisect

        def try_profile(P):
            # pool: desc list of (ntiles, expert, start)
            pool = sorted(((t, e, 0) for e, t in enumerate(tiles)),
                          reverse=True)
            bins = [[] for _ in range(NCORES)]
            used = [set() for _ in range(NCORES)]
            for pr in P:
                row = []
                for _ in range(NCORES):
                    if not pool:
                        row.append(None)
                        continue
                    t, e, st = pool.pop(0)
                    take = min(t, pr)
                    row.append((e, st, take))
                    if t > take:
                        item = (t - take, e, st + take)
                        bisect.insort(pool, item)
                        pool.reverse()
                        pool.sort(reverse=True)
                # assign row pieces to bins, avoiding same-expert repeats
                order = sorted(range(NCORES),
                               key=lambda c: sum(p[2] for p in bins[c]))
                pieces = [p for p in row if p is not None]
                taken = [False] * NCORES
                ok = True
                for p in pieces:
                    placed = False
                    for c in order:
                        if not taken[c] and p[0] not in used[c]:
                            bins[c].append(p)
                            used[c].add(p[0])
                            taken[c] = True
                            placed = True
                            break
                    if not placed:
                        ok = False
                        break
                if not ok:
                    return None
            if pool:
                return None
            for b in bins:
                b.sort(key=lambda p: -p[2])
            return bins

        for S in range(Q, Q + 3):
            for nlen in range(math.ceil(S / maxpart), GP + 1):
                for P in partitions(S, maxpart, nlen):
                    if len(P) != nlen:
                        continue
                    bins = try_profile(P)
                    if bins is not None:
                        return bins
        return None

    cands = []
    pc = profile_construct()
    if pc is not None and all(valid(b) and len(b) <= GP for b in pc):
        cands.append(pc)
    for base in (lpt(), split_fill()):
        if all(valid(b) and len(b) <= GP for b in base):
            cands.append(base)
            cands.append(local_search(base))
    bins = min(cands, key=cost_of)
    G = max(len(b) for b in bins)
    profile = tuple(
        max((b[r][2] if r < len(b) else 0) for b in bins) for r in range(G))
    bins = [list(b) + [(-1, 0, 0)] * (G - len(b)) for b in bins]
    return profile, bins


# --------------------------------------------------------------------------
# device program (structure depends only on `profile`)
# --------------------------------------------------------------------------

def _build_program(profile):
    G = len(profile)
    S8 = [16 * tc for tc in profile]    # per-channel columns (8-lane wrap)
    S16 = [8 * tc for tc in profile]    # idxw columns (16-lane gather wrap)
    S8MAX = max(S8)
    SWT = sum(S16)

    nc = bacc.Bacc("TRN2", target_bir_lowering=False, debug=False,
                   enable_asserts=False)

    xthl_in = nc.dram_tensor("xthl", [NSTRIP, 128, 2, HC, SW], BF,
                             kind="ExternalInput")
    xbf_in = nc.dram_tensor("xbfh", [TP, H], BF, kind="ExternalInput")
    gwh_in = nc.dram_tensor("gwh", [128, HC, E], BF, kind="ExternalInput")
    gwl_in = nc.dram_tensor("gwl", [128, HC, E], BF, kind="ExternalInput")
    bias_in = nc.dram_tensor("biasb", [128, E], F32, kind="ExternalInput")
    idf_in = nc.dram_tensor("identf", [128, 128], F32, kind="ExternalInput")
    idb_in = nc.dram_tensor("identb", [128, 128], BF, kind="ExternalInput")
    dat_in = nc.dram_tensor("dat16", [128, T], I16, kind="ExternalInput")
    e8_in = nc.dram_tensor("e8", [16, 128], F32R, kind="ExternalInput")
    rev_in = nc.dram_tensor("rev", [128, G, 128], F32, kind="ExternalInput")
    rod_in = nc.dram_tensor("rod", [128, G, 128], F32, kind="ExternalInput")
    labs_in = nc.dram_tensor("labs", [128, 1], F32, kind="ExternalInput")
    lthr_in = nc.dram_tensor("lthr", [128, 1], F32, kind="ExternalInput")
    loff_in = nc.dram_tensor("loff", [128, 1], F32, kind="ExternalInput")
    wgu_in = nc.dram_tensor("wgu", [G, 128, HC, 2 * II], BF, kind="ExternalInput")
    wd_in = nc.dram_tensor("wd", [G, 128, IC, H], BF, kind="ExternalInput")

    gatS = nc.dram_tensor("gatS", [TP, E], F32, kind="Internal")
    pout = nc.dram_tensor("pout", [TP, H], BF, kind="ExternalOutput")

    xbf_ap = xbf_in.ap()
    gatS_ap = gatS.ap()
    pout_ap = pout.ap()

    with tile.TileContext(nc) as tc:
        with tc.tile_pool(name="const", bufs=1) as cp, \
             tc.tile_pool(name="pwg", bufs=2) as pwg, \
             tc.tile_pool(name="pwd", bufs=1) as pwd:
            identf = cp.tile([128, 128], F32)
            nc.scalar.dma_start(identf[:], idf_in.ap())
            identb = cp.tile([128, 128], BF)
            nc.scalar.dma_start(identb[:], idb_in.ap())
            gwh = cp.tile([128, HC, E], BF)
            nc.scalar.dma_start(gwh[:], gwh_in.ap())
            gwl = cp.tile([128, HC, E], BF)
            nc.scalar.dma_start(gwl[:], gwl_in.ap())
            bias_s = cp.tile([128, E], F32)
            nc.scalar.dma_start(bias_s[:], bias_in.ap())
            idxw = cp.tile([128, SWT], I16)
            # slot-0 weights from the persistent pools, DMA'd behind the
            # strips at the end of P1 (no SBUF overlap with P1/P2 pools ->
            # no WAR delay)
            wgus0 = pwg.tile([128, HC, 2 * II], BF, tag="wgu", name="wgup0")
            wds0 = pwd.tile([128, IC, H], BF, tag="wd", name="wdp0")
            ll1 = nc.gpsimd.load_library(library_config.local_scatter)

          # (indentation shim)
          # P1/P2-scoped constants
            p12_cm = tc.tile_pool(name="p12", bufs=1)
            p12 = p12_cm.__enter__()
            dat16 = p12.tile([128, T], I16)
            e8 = p12.tile([16, 128], F32R)
            rev = p12.tile([128, G, 128], F32)
            rod = p12.tile([128, G, 128], F32)
            labs = p12.tile([128, 1], F32)
            lthr = p12.tile([128, 1], F32)
            loff = p12.tile([128, 1], F32)
            zf = p12.tile([16, E], F32)
            nc.vector.memset(zf[:], 0.0)
            gts = [p12.tile([16, 512], BF, name=f"gts{k}")
                   for k in range(T // 512)]
            # P2 chain tiles, allocated early so they never overlap the P1
            # pools' SBUF (overlap would add a WAR wait for all of P1)
            mbt = p12.tile([16, 512], F32, name="mbt")
            csA = p12.tile([16, 512], F32, name="csA")
            csB = p12.tile([16, 512], F32, name="csB")
            qht = p12.tile([16, 512], F32R, name="qht")
            abt = p12.tile([128, 512], F32, name="abt")
            cct = p12.tile([128, 512], F32, name="cct")
            t1t = p12.tile([128, 512], F32, name="t1t")
            idx16 = p12.tile([128, T], I16, name="idx16")
            lists = p12.tile([128, S8MAX], I16, name="lists")
            lf = p12.tile([128, S8MAX], F32, name="lf")

            strip_dmas = []

            # ---------------- P1: router ----------------
            # Strip DMA order h0,h1,l0,h2,l1,...,h7,l6,l7 on the ACT queue;
            # per chunk the xh@gh + xh@gl terms run at hi-strip arrival and
            # the xl@gh term (plus the gating chain) one strip behind, so
            # PE/DVE work overlaps the strip loads.
            with tc.tile_pool(name="p1xh", bufs=2) as p1xh, \
                 tc.tile_pool(name="p1xl", bufs=2) as p1xl, \
                 tc.tile_pool(name="p1s", bufs=2) as p1s, \
                 tc.tile_pool(name="p1g", bufs=4) as p1g, \
                 tc.tile_pool(name="p1pl", bufs=3, space="PSUM") as p1pl, \
                 tc.tile_pool(name="p1p8", bufs=2, space="PSUM") as p1p8, \
                 tc.tile_pool(name="p2pb", bufs=2, space="PSUM") as p2pb, \
                 tc.tile_pool(name="p2pr", bufs=1, space="PSUM") as p2pr:

                CPS = SW // 128  # chunks per strip

                def load_strip(st):
                    xs = p1xh.tile([128, 2, HC, SW], BF, tag="xhl",
                                   name=f"xs{st}")
                    if st < 1:
                        # halved: faster first arrival (512B descriptors
                        # still run at full DMA bandwidth; smaller would
                        # halve it)
                        for q2 in range(2):
                            qs = slice(q2 * (SW // 2), (q2 + 1) * (SW // 2))
                            strip_dmas.append(nc.sync.dma_start(
                                xs[:, :, :, qs],
                                xthl_in.ap()[st][:, :, :, qs]))
                    else:
                        strip_dmas.append(
                            nc.sync.dma_start(xs[:], xthl_in.ap()[st]))
                    return xs

                def router_chunk(c, xs):
                    cc = c % CPS
                    csl = slice(cc * 128, (cc + 1) * 128)
                    lg = p1pl.tile([128, E], F32, tag="lg", name=f"lg{c}")
                    for hc in range(HC):
                        nc.tensor.matmul(lg[:], lhsT=xs[:, 0, hc, csl],
                                         rhs=gwh[:, hc, :],
                                         start=(hc == 0), stop=False)
                    for hc in range(HC):
                        nc.tensor.matmul(lg[:], lhsT=xs[:, 0, hc, csl],
                                         rhs=gwl[:, hc, :],
                                         start=False, stop=False)
                    for hc in range(HC):
                        nc.tensor.matmul(lg[:], lhsT=xs[:, 1, hc, csl],
                                         rhs=gwh[:, hc, :],
                                         start=False, stop=(hc == HC - 1))
                    return lg

                def gating_chunk(c, lg):
                    rows = slice(c * 128, (c + 1) * 128)
                    sc = p1s.tile([128, E], F32, tag="sc", name=f"sc{c}")
                    nc.scalar.activation(sc[:], lg[:], AF.Sigmoid)
                    sel = p1s.tile([128, E], F32, tag="sel", name=f"se{c}")
                    nc.vector.tensor_add(sel[:], sc[:], bias_s[:])
                    mx8 = p1s.tile([128, 8], F32, tag="mx8", name=f"mx{c}")
                    nc.vector.max(out=mx8[:], in_=sel[:])
                    msel = p1s.tile([128, E], F32, tag="msel", name=f"ms{c}")
                    nc.vector.match_replace(out=msel[:], in_to_replace=mx8[:],
                                            in_values=sel[:], imm_value=-1e30)
                    maskc = p1s.tile([128, E], F32, tag="maskc", name=f"mc{c}")
                    nc.vector.tensor_scalar(maskc[:], msel[:], -1e29, None,
                                            op0=ALU.is_le)
                    wm = p1s.tile([128, E], F32, tag="wm", name=f"wm{c}")
                    ssum = p1s.tile([128, 1], F32, tag="ssum", name=f"ss{c}")
                    nc.vector.scalar_tensor_tensor(out=wm[:], in0=sc[:],
                                                   scalar=0.0, in1=maskc[:],
                                                   op0=ALU.add, op1=ALU.mult,
                                                   accum_out=ssum[:])
                    winv = p1s.tile([128, 1], F32, tag="winv", name=f"wv{c}")
                    nc.vector.reciprocal(winv[:], ssum[:])
                    gt = p1g.tile([128, E], F32, tag="gt", name=f"gt{c}")
                    nc.vector.tensor_scalar_mul(gt[:], wm[:], winv[:])
                    # SP queue, interleaved behind the strips: by the time
                    # the next strip wants to issue, this chunk's gt is
                    # long ready, so no head-of-line risk; and the small
                    # transfers slip between strip transfers in the pool.
                    nc.sync.dma_start(gatS_ap[rows, :], gt[:])
                    tp16 = p1p8.tile([128, 128], F32, tag="tp16")
                    nc.tensor.transpose(tp16[:GP, :], gt[:, 0:GP], identf[:])
                    gdst = gts[c // 4]
                    gcol0 = (c % 4) * 128
                    nc.scalar.activation(gdst[0:GP, gcol0:gcol0 + 128],
                                         tp16[:GP, :], AF.Copy)

                lgs = {}
                strips = {}
                for c in range(NCHUNK + 1):
                    if c < NCHUNK:
                        st = c // CPS
                        if c % CPS == 0:
                            strips[st] = load_strip(st)
                        lgs[c] = router_chunk(c, strips[st])
                    if c >= 1:
                        gating_chunk(c - 1, lgs.pop(c - 1))
                nc.sync.dma_start(gatS_ap[T:TP, :], zf[:])
                # slot-0 weights: explicitly ordered after the last strips
                # (the scheduler would otherwise hoist these dep-free DMAs
                # into the middle of the strip sequence)
                wg0d = nc.scalar.dma_start(wgus0[:], wgu_in.ap()[0])
                wd0d = nc.scalar.dma_start(wds0[:], wd_in.ap()[0])
                _add_dep_helper(wg0d.ins, strip_dmas[-1].ins, False,
                                "wgu0 after strips")

            # ---------------- P2: dispatch ----------------
                for _t, _src in ((dat16, dat_in), (e8, e8_in),
                                 (rev, rev_in), (rod, rod_in),
                                 (labs, labs_in), (lthr, lthr_in),
                                 (loff, loff_in)):
                    _d = nc.scalar.dma_start(_t[:], _src.ap())
                    _add_dep_helper(_d.ins, strip_dmas[-1].ins, False,
                                    "p2 consts after strips")

                # segmented mask/scan/window chain (512 tokens per segment):
                # runs concurrently with P1 as gTS columns land.
                csprev = None
                for sk in range(T // 512):
                        hf, nt = sk // 4, sk % 4
                        nc.vector.tensor_scalar(mbt[:], gts[sk][:], 0.0,
                                                None, op0=ALU.is_gt)
                        cs = csA if sk % 2 == 0 else csB
                        ini = 0.0 if csprev is None else csprev[:, 511:512]
                        nc.vector.tensor_tensor_scan(cs[:], data0=mbt[:],
                                                     data1=mbt[:],
                                                     initial=ini,
                                                     op0=ALU.add,
                                                     op1=ALU.bypass)
                        csprev = cs
                        nc.vector.tensor_mul(qht[:], cs[:], mbt[:])
                        # lane ch=8s+p: valid slot idx = (q-1)-start_s-S8_s*p
                        # iff in [0, S8_s): bp = q; ab = |bp + labs|;
                        # cc = ab <= lthr; idx16 = (bp + loff)*cc - 1.
                        bp = p2pb.tile([128, 512], F32, tag="bp")
                        nc.tensor.matmul(bp[:], lhsT=e8[:, :], rhs=qht[:],
                                         start=True, stop=True)
                        nc.scalar.activation(abt[:], bp[:], AF.Abs,
                                             bias=labs[:])
                        nc.vector.tensor_scalar(cct[:], abt[:], lthr[:],
                                                None, op0=ALU.is_le)
                        nc.vector.scalar_tensor_tensor(
                            out=t1t[:], in0=bp[:], scalar=loff[:], in1=cct[:],
                            op0=ALU.add, op1=ALU.mult)
                        col = hf * TH + nt * 512
                        nc.vector.tensor_scalar_add(idx16[:, col:col + 512],
                                                    t1t[:], -1.0)

                lsc = nc.gpsimd.local_scatter(out_ap=lists[:],
                                              data_ap=dat16[:],
                                              idxs_ap=idx16[:], channels=128,
                                              num_elems=S8MAX, num_idxs=T)
                ll2 = nc.gpsimd.load_library(library_config.mlp)
                _add_dep_helper(lsc.ins, ll1.ins, True,
                                "lib order: ls after load7")
                _add_dep_helper(ll2.ins, lsc.ins, True,
                                "lib order: load3 after ls")
                # wd0 transfer would jam the DMA pool right when the first
                # x-gathers need it; not needed until the first stage_D
                _add_dep_helper(wd0d.ins, lsc.ins, False, "wd0 after ls")

                nc.vector.tensor_copy(lf[:], lists[:])
                # re-wrap 8-lane channels into the 16-lane gather layout:
                # idxw[row, s, c] = lists[8s + (row%16)//2, c + S16_s*(row%2)]
                # (+T so empty slots (0) hit the zero-row sentinel)
                off = 0
                for s in range(G):
                    if profile[s] == 0:
                        continue
                    rp = p2pr.tile([128, 512], F32, tag="rp")
                    nc.tensor.matmul(rp[:, 0:S16[s]], lhsT=rev[:, s, :],
                                     rhs=lf[:, 0:S16[s]],
                                     start=True, stop=False)
                    nc.tensor.matmul(rp[:, 0:S16[s]], lhsT=rod[:, s, :],
                                     rhs=lf[:, S16[s]:S8[s]],
                                     start=False, stop=True)
                    nc.vector.tensor_scalar_add(idxw[:, off:off + S16[s]],
                                                rp[:, 0:S16[s]], float(T))
                    off += S16[s]

            p12_cm.__exit__(None, None, None)

            # ---------------- P3: expert SwiGLU GEMMs ----------------
            # software pipeline: per row-tile, stage A = g/u matmuls (two
            # halves), B = silu chain (ACT/DVE), C = PE transposes of h,
            # D = down matmuls + scale + scatter.  Emission order puts
            # C(i-1) between A's two halves and D(i-1) after A(i) so the
            # in-order PE queue never stalls on the B/C copies.
            swdge = []
            with tc.tile_pool(name="px", bufs=2) as px, \
                 tc.tile_pool(name="pgg", bufs=2) as pgg, \
                 tc.tile_pool(name="pa", bufs=3) as pa, \
                 tc.tile_pool(name="psG", bufs=4, space="PSUM") as psG, \
                 tc.tile_pool(name="psT", bufs=2, space="PSUM") as psT, \
                 tc.tile_pool(name="psY", bufs=2, space="PSUM") as psY:
                HW2 = II // 2  # 384

                tiles_list = []       # (slot, rt, xte, rsl, wgus, wds, ggat)
                off = 0
                for s in range(G):
                    TC = profile[s]
                    if TC == 0:
                        continue
                    iws = idxw[:, off:off + S16[s]]
                    off += S16[s]
                    if s == 0:
                        wgus, wds = wgus0, wds0
                    else:
                        wgus = pwg.tile([128, HC, 2 * II], BF, tag="wgu")
                        d1 = nc.scalar.dma_start(wgus[:], wgu_in.ap()[s])
                        wds = pwd.tile([128, IC, H], BF, tag="wd")
                        d2 = nc.scalar.dma_start(wds[:], wd_in.ap()[s])
                        if s == 1:
                            _add_dep_helper(d1.ins, lsc.ins, False,
                                            "wgu1 after ls")
                            _add_dep_helper(d2.ins, lsc.ins, False,
                                            "wd1 after ls")
                    ggat = pgg.tile([128, TC, E], F32, tag="gg")
                    for g0 in range(0, TC, 4):
                        gn = min(4, TC - g0)
                        rn = gn * 128
                        xte = px.tile([128, HC, rn], BF, tag="xt")
                        g2 = nc.gpsimd.dma_gather(
                            out_ap=xte[:], in_ap=xbf_ap[:],
                            idxs_ap=iws[:, g0 * 8:(g0 + gn) * 8],
                            num_idxs=rn, num_idxs_reg=rn, elem_size=H,
                            transpose=True)
                        swdge.append(g2)
                        if g0 == 0:
                            # gating gather after the first x-gather: it is
                            # only needed at stage D, keep it off the
                            # critical path to the first matmuls.
                            g1 = nc.gpsimd.dma_gather(
                                out_ap=ggat[:], in_ap=gatS_ap[:],
                                idxs_ap=iws,
                                num_idxs=TC * 128, num_idxs_reg=TC * 128,
                                elem_size=E)
                            swdge.append(g1)
                        for rti in range(gn):
                            rt = g0 + rti
                            rsl = slice(rti * 128, (rti + 1) * 128)
                            tiles_list.append((s, rt, xte, rsl, wgus, wds,
                                               ggat, iws))

                def stage_A(i, half2):
                    s, rt, xte, rsl, wgus, wds, ggat, iws = tiles_list[i]
                    io = half2 * HW2
                    gph = psG.tile([128, HW2], F32, tag="gu",
                                   name=f"gp{i}_{half2}")
                    uph = psG.tile([128, HW2], F32, tag="gu",
                                   name=f"up{i}_{half2}")
                    for hc in range(HC):
                        for ps, io2 in ((gph, io), (uph, II + io)):
                            nc.tensor.matmul(
                                ps[:], lhsT=xte[:, hc, rsl],
                                rhs=wgus[:, hc, io2:io2 + HW2],
                                start=(hc == 0), stop=(hc == HC - 1))
                    return gph, uph

                def stage_B(i, half2, gph, uph, hT):
                    gsh = pa.tile([128, HW2], F32, tag="gs",
                                  name=f"gs{i}_{half2}")
                    nc.scalar.activation(gsh[:], gph[:], AF.Sigmoid)
                    m1h = pa.tile([128, HW2], F32, tag="m1",
                                  name=f"m1{i}_{half2}")
                    nc.vector.tensor_mul(m1h[:], gsh[:], gph[:])
                    hbh = pa.tile([128, HW2], BF, tag="hbf",
                                  name=f"hb{i}_{half2}")
                    nc.vector.tensor_mul(hbh[:], m1h[:], uph[:])
                    return hbh

                def stage_C(i, half2, hbh, hT):
                    tp = psT.tile([128, 3, 128], BF, tag="tp")
                    for ici in range(IC // 2):
                        nc.tensor.transpose(
                            tp[:, ici, :],
                            hbh[:, ici * 128:(ici + 1) * 128],
                            identb[:])
                    i0 = half2 * (IC // 2)
                    if half2 == 0:
                        nc.vector.tensor_copy(hT[:, i0:i0 + 3, :], tp[:])
                    else:
                        nc.scalar.activation(hT[:, i0:i0 + 3, :], tp[:],
                                             AF.Copy)

                def stage_D(i, hT):
                    s, rt, xte, rsl, wgus, wds, ggat, iws = tiles_list[i]
                    ysc = pa.tile([128, 1, H], BF, tag="ysc", name=f"ys{i}")
                    gcol = ggat[:, rt, s:s + 1]
                    for n3 in range(3):
                        yp = psY.tile([128, 512], F32, tag="y")
                        for ic in range(IC):
                            nc.tensor.matmul(
                                yp[:], lhsT=hT[:, ic, :],
                                rhs=wds[:, ic, n3 * 512:(n3 + 1) * 512],
                                start=(ic == 0), stop=(ic == IC - 1))
                        nc.vector.tensor_scalar_mul(
                            ysc[:, 0, n3 * 512:(n3 + 1) * 512], yp[:], gcol)
                    s1 = nc.gpsimd.dma_scatter_add(
                        out_ap=pout_ap[:], in_ap=ysc[:],
                        idxs_ap=iws[:, rt * 8:rt * 8 + 8],
                        num_idxs=128, num_idxs_reg=128, elem_size=H)
                    swdge.append(s1)

                NT = len(tiles_list)
                state = {}   # i -> (hbh0, hbh1, hT)
                for i in range(NT + 1):
                    if i < NT:
                        hT = pa.tile([128, IC, 128], BF, tag="hT",
                                     name=f"hT{i}")
                        g0, u0 = stage_A(i, 0)
                        hb0 = stage_B(i, 0, g0, u0, hT)
                        if i >= 1:
                            hb0p, hb1p, hTp = state.pop(i - 1)
                            stage_C(i - 1, 0, hb0p, hTp)
                        g1_, u1_ = stage_A(i, 1)
                        hb1 = stage_B(i, 1, g1_, u1_, hT)
                        if i >= 1:
                            stage_C(i - 1, 1, hb1p, hTp)
                            stage_D(i - 1, hTp)
                        state[i] = (hb0, hb1, hT)
                    else:
                        hb0p, hb1p, hTp = state.pop(i - 1)
                        stage_C(i - 1, 0, hb0p, hTp)
                        stage_C(i - 1, 1, hb1p, hTp)
                        stage_D(i - 1, hTp)
            for ins in swdge:
                _add_dep_helper(ins.ins, ll2.ins, False,
                                "lib order: mlp ops after load3")

    nc.compile()
    return nc


_NC_CACHE = {}


def _get_program(profile):
    if profile not in _NC_CACHE:
        _NC_CACHE[profile] = _build_program(profile)
    return _NC_CACHE[profile]


# --------------------------------------------------------------------------
# host-side input prep
# --------------------------------------------------------------------------

def _split_bf16(a):
    hi = a.astype(BF16)
    lo = (a - hi.astype(np.float32)).astype(BF16)
    return hi, lo


def make_in_maps(hidden_states, gate_w, routing_bias, w_gate, w_up, w_down,
                 profile, bins):
    G = len(profile)
    S8 = [16 * tc for tc in profile]
    S16 = [8 * tc for tc in profile]

    x = np.asarray(hidden_states, dtype=np.float32)
    gw = np.asarray(gate_w, dtype=np.float32)
    rb = np.asarray(routing_bias, dtype=np.float32)
    wgt = np.asarray(w_gate)
    wut = np.asarray(w_up)
    wdt = np.asarray(w_down)

    xh, xl = _split_bf16(x)
    xbf = np.zeros((TP, H), dtype=BF16)
    xbf[:T] = xh

    def strips(a):
        # [T, H] -> [NSTRIP, 128, HC, SW];  [st, p, hc, t] = a[st*SW+t, hc*128+p]
        return np.ascontiguousarray(
            a.reshape(NSTRIP, SW, HC, 128).transpose(0, 3, 2, 1))

    xthl = np.ascontiguousarray(
        np.stack([strips(xh), strips(xl)], axis=2))

    gwh32 = gw.astype(BF16).astype(np.float32)
    gwl32 = (gw - gwh32).astype(BF16).astype(np.float32)

    identf = np.eye(128, dtype=np.float32)
    identb = np.eye(128).astype(BF16)
    dat16 = np.tile(np.arange(-T, 0, dtype=np.int16), (128, 1))
    e8 = np.zeros((16, 128), np.float32)
    for s in range(min(G, 16)):
        e8[s, 8 * s:8 * s + 8] = 1.0
    rev = np.zeros((128, G, 128), np.float32)
    rod = np.zeros((128, G, 128), np.float32)
    for s in range(G):
        for row in range(128):
            q = row % 16
            ch = 8 * s + q // 2
            if row % 2 == 0:
                rev[ch, s, row] = 1.0
            else:
                rod[ch, s, row] = 1.0

    def gwtr(a):
        # [E, H] (fp32) -> [128, HC, E] bf16
        return np.ascontiguousarray(
            a.T.reshape(HC, 128, E).transpose(1, 0, 2)).astype(BF16)

    in_maps = []
    for c in range(NCORES):
        pieces = bins[c]
        slot_exp = [p[0] for p in pieces]
        used = set(e for e in slot_exp if e >= 0)
        rest = [e for e in range(E) if e not in used]
        ri = 0
        perm = []
        for e in slot_exp:
            if e >= 0:
                perm.append(e)
            else:
                perm.append(rest[ri])
                ri += 1
        perm += rest[ri:]
        assert sorted(perm) == list(range(E))
        perm = np.array(perm)

        labs = np.zeros((128, 1), np.float32)
        lthr = np.full((128, 1), -1.0, np.float32)
        loff = np.zeros((128, 1), np.float32)
        for s in range(G):
            e, st_tile, ntiles = pieces[s]
            if e < 0 or ntiles == 0:
                continue
            start = st_tile * 128
            for p in range(8):
                ch = 8 * s + p
                base = start + S8[s] * p
                labs[ch] = -(base + 1) - (S8[s] - 1) / 2.0
                lthr[ch] = (S8[s] - 1) / 2.0
                loff[ch] = -base

        wgu = np.zeros((G, 128, HC, 2 * II), BF16)
        wd = np.zeros((G, 128, IC, H), BF16)
        for s in range(G):
            e = pieces[s][0]
            if e < 0 or profile[s] == 0:
                continue
            wg_t = wgt[e].T.reshape(HC, 128, II).transpose(1, 0, 2)
            wu_t = wut[e].T.reshape(HC, 128, II).transpose(1, 0, 2)
            wgu[s][:, :, :II] = wg_t.astype(BF16)
            wgu[s][:, :, II:] = wu_t.astype(BF16)
            wd[s] = wdt[e].T.reshape(IC, 128, H).transpose(1, 0, 2).astype(BF16)

        in_maps.append(dict(
            xthl=xthl, xbfh=xbf,
            gwh=gwtr(gwh32[perm]),
            gwl=gwtr(gwl32[perm]),
            biasb=np.tile(rb[perm][None, :], (128, 1)).astype(np.float32),
            identf=identf, identb=identb, dat16=dat16, e8=e8,
            rev=rev, rod=rod, labs=labs, lthr=lthr, loff=loff,
            wgu=wgu, wd=wd,
        ))
    return in_maps


def kernel(hidden_states, gate_w, routing_bias, w_gate, w_up, w_down,
           num_global_tokens=None, max_num_tokens_per_gpu=None, **_unused):
    x = np.asarray(hidden_states, np.float32)
    gw = np.asarray(gate_w, np.float32)
    rb = np.asarray(routing_bias, np.float32)
    loads = _host_loads(x, gw, rb)
    profile, bins = _schedule(loads)
    nc = _get_program(profile)
    in_maps = make_in_maps(x, gw, rb, w_gate, w_up, w_down, profile, bins)
    res = bass_utils.run_bass_kernel_spmd(nc, in_maps,
                                          core_ids=list(range(NCORES)))
    out = np.zeros((T, H), dtype=np.float32)
    for c in range(NCORES):
        out += np.asarray(res.results[c]["pout"])[:T].astype(np.float32)
    return out


# revision 32
# speedup vs baseline: 1.0280x; 1.0280x over previous
"""MiniMax-M2 sparse MoE block on 8 Trainium2 NeuronCores (expert-parallel).

v2: load-balanced adaptive schedule + split-precision router.

T=4096 tokens, H=1536, I=768, E=64 experts, top-8 sigmoid routing.

Key ideas vs v1 (8 static experts/core x 7 capacity tiles = 56 row-tiles):
  * PE matmul cost scales with the output free-dim only, so P3 cost =
    (#row-tiles) x ~11.8us.  The per-core minimum is
    ceil(sum_e ceil(load_e/128) / 8) ~= 38 tiles.  A host-side scheduler
    (the router is recomputed on host in fp32 purely to pick the schedule;
    all math still runs on device) bin-packs experts into 8 cores with an
    identical per-slot tile profile, splitting experts across cores when
    needed.
  * Split-precision router: x = xh + xl (bf16 each), gw = gh + gl;
    logits = xh@gh + xl@gh + xh@gl accumulated in fp32 PSUM.  Zero top-8
    flips vs fp32 (logit err ~1.6e-5) at bf16 matmul speed.  The
    transposed x operands are host-prepared, so P1 needs no PE transposes.
  * Dispatch (P2) generalized to per-slot (expert, start, ntiles) pieces;
    all per-slot binding constants are input data, only the slot
    tile-count profile is program structure (identical across cores ->
    SPMD-safe).  local_scatter uses 8-lane channels (16 slots max in one
    128-channel page); two matmuls per slot re-wrap the lists into the
    16-lane replicated layout the SWDGE gathers expect.
  * pout accumulates in bf16 (halves scatter-add traffic); host sums the
    8 partials in fp32.
"""

import math

import numpy as np
import ml_dtypes

import concourse.bass as bass
import concourse.mybir as mybir
import concourse.tile as tile
from concourse import bacc, library_config
from concourse import bass_utils
from concourse.bass import _add_dep_helper

BF16 = ml_dtypes.bfloat16

T = 4096
H = 1536
II = 768
E = 64
K = 8
NCORES = 8
PAD = 16          # safety margin on host-predicted expert loads
TP = T + 16       # padded token rows; rows T.. are the zero sentinel
AF = mybir.ActivationFunctionType
ALU = mybir.AluOpType
F32 = mybir.dt.float32
F32R = mybir.dt.float32r
BF = mybir.dt.bfloat16
I16 = mybir.dt.int16
HC = H // 128      # 12
IC = II // 128     # 6
NCHUNK = T // 128  # 32
TH = T // 2
GP = 16            # gatS width / max slots
NSTRIP = 8
SW = T // NSTRIP   # 512


# --------------------------------------------------------------------------
# host-side routing + scheduling
# --------------------------------------------------------------------------

def _host_loads(x, gw, rb):
    """fp32 router on host; per-expert token counts (schedule only)."""
    logits = x.astype(np.float32) @ gw.astype(np.float32).T
    scores = 1.0 / (1.0 + np.exp(-logits))
    sel = scores + rb.astype(np.float32)[None, :]
    kth = np.partition(sel, E - K, axis=1)[:, E - K][:, None]
    return (sel >= kth).sum(axis=0).astype(int)


def _schedule(loads):
    """Bin-pack experts into 8 cores x G slots.

    Returns (profile, bins): profile = per-slot tile counts (descending,
    shared across cores); bins[c] = list of (expert, start_tile, ntiles)
    sorted descending by ntiles, padded with (-1, 0, 0) to len(profile).
    No expert appears twice in one bin.
    """
    import random
    tiles = [max(1, math.ceil((l + PAD) / 128)) for l in loads]
    TT = sum(tiles)
    Q = math.ceil(TT / NCORES)

    def rankmax(bins):
        g = max(len(b) for b in bins)
        return sum(max((b[r][2] if r < len(b) else 0) for b in bins)
                   for r in range(g))

    def valid(b):
        return len({p[0] for p in b}) == len(b)

    def lpt():
        order = sorted(range(E), key=lambda e: -tiles[e])
        bins = [[] for _ in range(NCORES)]
        sums = [0] * NCORES
        for e in order:
            c = min(range(NCORES), key=lambda i: (sums[i], len(bins[i])))
            bins[c].append((e, 0, tiles[e]))
            sums[c] += tiles[e]
        for b in bins:
            b.sort(key=lambda p: -p[2])
        return bins

    def split_fill():
        order = sorted(range(E), key=lambda e: -tiles[e])
        bins = [[] for _ in range(NCORES)]
        c, rem = 0, Q
        for e in order:
            t, st = tiles[e], 0
            while t > 0:
                if rem == 0 and c < NCORES - 1:
                    c += 1
                    rem = Q
                take = min(t, rem)
                bins[c].append((e, st, take))
                st += take
                t -= take
                rem -= take
        for b in bins:
            b.sort(key=lambda p: -p[2])
        return bins

    rng = random.Random(0)

    def cost_of(bins):
        return rankmax(bins) * 100 + max(len(b) for b in bins)

    def local_search(base, iters=20000):
        bins = [list(b) for b in base]
        best = cost_of(bins)
        for _ in range(iters):
            c1, c2 = rng.randrange(NCORES), rng.randrange(NCORES)
            if c1 == c2 or not bins[c1]:
                continue
            i1 = rng.randrange(len(bins[c1]))
            saved = (list(bins[c1]), list(bins[c2]))
            r = rng.random()
            if r < 0.4 and bins[c2]:
                i2 = rng.randrange(len(bins[c2]))
                bins[c1][i1], bins[c2][i2] = bins[c2][i2], bins[c1][i1]
            elif r < 0.7:
                bins[c2].append(bins[c1].pop(i1))
            else:
                e, st, nt = bins[c1][i1]
                if nt < 2:
                    continue
                k = rng.randrange(1, nt)
                bins[c1][i1] = (e, st, nt - k)
                bins[c2].append((e, st + nt - k, k))
            if valid(bins[c1]) and valid(bins[c2]) and \
                    max(len(bins[c1]), len(bins[c2])) <= GP:
                for b in (bins[c1], bins[c2]):
                    b.sort(key=lambda p: -p[2])
                cost = cost_of(bins)
                if cost <= best:
                    best = cost
                    continue
            bins[c1], bins[c2] = saved
        return bins

    def profile_construct():
        """Enumerate slot profiles (desc partitions of S, parts<=7) in
        (S, nslots) order; rank-deal items into 8 bins with splitting;
        return bins for the first feasible profile."""
        maxpart = min(7, max(tiles))

        def partitions(S, maxp, maxlen):
            if S == 0:
                yield []
                return
            if maxlen == 0:
                return
            for p in range(min(maxp, S), 0, -1):
                for rest in partitions(S - p, p, maxlen - 1):
                    yield [p] + rest

        import bisect

        def try_profile(P):
            # pool: desc list of (ntiles, expert, start)
            pool = sorted(((t, e, 0) for e, t in enumerate(tiles)),
                          reverse=True)
            bins = [[] for _ in range(NCORES)]
            used = [set() for _ in range(NCORES)]
            for pr in P:
                row = []
                for _ in range(NCORES):
                    if not pool:
                        row.append(None)
                        continue
                    t, e, st = pool.pop(0)
                    take = min(t, pr)
                    row.append((e, st, take))
                    if t > take:
                        item = (t - take, e, st + take)
                        bisect.insort(pool, item)
                        pool.reverse()
                        pool.sort(reverse=True)
                # assign row pieces to bins, avoiding same-expert repeats
                order = sorted(range(NCORES),
                               key=lambda c: sum(p[2] for p in bins[c]))
                pieces = [p for p in row if p is not None]
                taken = [False] * NCORES
                ok = True
                for p in pieces:
                    placed = False
                    for c in order:
                        if not taken[c] and p[0] not in used[c]:
                            bins[c].append(p)
                            used[c].add(p[0])
                            taken[c] = True
                            placed = True
                            break
                    if not placed:
                        ok = False
                        break
                if not ok:
                    return None
            if pool:
                return None
            for b in bins:
                b.sort(key=lambda p: -p[2])
            return bins

        for S in range(Q, Q + 3):
            for nlen in range(math.ceil(S / maxpart), GP + 1):
                for P in partitions(S, maxpart, nlen):
                    if len(P) != nlen:
                        continue
                    bins = try_profile(P)
                    if bins is not None:
                        return bins
        return None

    cands = []
    pc = profile_construct()
    if pc is not None and all(valid(b) and len(b) <= GP for b in pc):
        cands.append(pc)
    for base in (lpt(), split_fill()):
        if all(valid(b) and len(b) <= GP for b in base):
            cands.append(base)
            cands.append(local_search(base))
    bins = min(cands, key=cost_of)
    G = max(len(b) for b in bins)
    profile = tuple(
        max((b[r][2] if r < len(b) else 0) for b in bins) for r in range(G))
    bins = [list(b) + [(-1, 0, 0)] * (G - len(b)) for b in bins]
    return profile, bins


# --------------------------------------------------------------------------
# device program (structure depends only on `profile`)
# --------------------------------------------------------------------------

def _build_program(profile):
    G = len(profile)
    S8 = [16 * tc for tc in profile]    # per-channel columns (8-lane wrap)
    S16 = [8 * tc for tc in profile]    # idxw columns (16-lane gather wrap)
    S8MAX = max(S8)
    SWT = sum(S16)

    nc = bacc.Bacc("TRN2", target_bir_lowering=False, debug=False,
                   enable_asserts=False)

    xthl_in = nc.dram_tensor("xthl", [NSTRIP, 128, 2, HC, SW], BF,
                             kind="ExternalInput")
    xbf_in = nc.dram_tensor("xbfh", [TP, H], BF, kind="ExternalInput")
    gwh_in = nc.dram_tensor("gwh", [128, HC, E], BF, kind="ExternalInput")
    gwl_in = nc.dram_tensor("gwl", [128, HC, E], BF, kind="ExternalInput")
    bias_in = nc.dram_tensor("biasb", [128, E], F32, kind="ExternalInput")
    idf_in = nc.dram_tensor("identf", [128, 128], F32, kind="ExternalInput")
    idb_in = nc.dram_tensor("identb", [128, 128], BF, kind="ExternalInput")
    dat_in = nc.dram_tensor("dat16", [128, T], I16, kind="ExternalInput")
    e8_in = nc.dram_tensor("e8", [16, 128], F32R, kind="ExternalInput")
    rev_in = nc.dram_tensor("rev", [128, G, 128], F32, kind="ExternalInput")
    rod_in = nc.dram_tensor("rod", [128, G, 128], F32, kind="ExternalInput")
    labs_in = nc.dram_tensor("labs", [128, 1], F32, kind="ExternalInput")
    lthr_in = nc.dram_tensor("lthr", [128, 1], F32, kind="ExternalInput")
    loff_in = nc.dram_tensor("loff", [128, 1], F32, kind="ExternalInput")
    wgu_in = nc.dram_tensor("wgu", [G, 128, HC, 2 * II], BF, kind="ExternalInput")
    wd_in = nc.dram_tensor("wd", [G, 128, IC, H], BF, kind="ExternalInput")

    gatS = nc.dram_tensor("gatS", [TP, E], F32, kind="Internal")
    pout = nc.dram_tensor("pout", [TP, H], BF, kind="ExternalOutput")

    xbf_ap = xbf_in.ap()
    gatS_ap = gatS.ap()
    pout_ap = pout.ap()

    with tile.TileContext(nc) as tc:
        with tc.tile_pool(name="const", bufs=1) as cp, \
             tc.tile_pool(name="pwg", bufs=2) as pwg, \
             tc.tile_pool(name="pwd", bufs=1) as pwd:
            identf = cp.tile([128, 128], F32)
            nc.scalar.dma_start(identf[:], idf_in.ap())
            identb = cp.tile([128, 128], BF)
            nc.scalar.dma_start(identb[:], idb_in.ap())
            gwh = cp.tile([128, HC, E], BF)
            nc.scalar.dma_start(gwh[:], gwh_in.ap())
            gwl = cp.tile([128, HC, E], BF)
            nc.scalar.dma_start(gwl[:], gwl_in.ap())
            bias_s = cp.tile([128, E], F32)
            nc.scalar.dma_start(bias_s[:], bias_in.ap())
            idxw = cp.tile([128, SWT], I16)
            # slot-0 weights from the persistent pools, DMA'd behind the
            # strips at the end of P1 (no SBUF overlap with P1/P2 pools ->
            # no WAR delay)
            wgus0 = pwg.tile([128, HC, 2 * II], BF, tag="wgu", name="wgup0")
            wds0 = pwd.tile([128, IC, H], BF, tag="wd", name="wdp0")
            ll1 = nc.gpsimd.load_library(library_config.local_scatter)

          # (indentation shim)
          # P1/P2-scoped constants
            p12_cm = tc.tile_pool(name="p12", bufs=1)
            p12 = p12_cm.__enter__()
            dat16 = p12.tile([128, T], I16)
            e8 = p12.tile([16, 128], F32R)
            rev = p12.tile([128, G, 128], F32)
            rod = p12.tile([128, G, 128], F32)
            labs = p12.tile([128, 1], F32)
            lthr = p12.tile([128, 1], F32)
            loff = p12.tile([128, 1], F32)
            zf = p12.tile([16, E], F32)
            nc.vector.memset(zf[:], 0.0)
            gts = [p12.tile([16, 512], BF, name=f"gts{k}")
                   for k in range(T // 512)]
            # P2 chain tiles, allocated early so they never overlap the P1
            # pools' SBUF (overlap would add a WAR wait for all of P1)
            mbt = p12.tile([16, 512], F32, name="mbt")
            csA = p12.tile([16, 512], F32, name="csA")
            csB = p12.tile([16, 512], F32, name="csB")
            qht = p12.tile([16, 512], F32R, name="qht")
            abt = p12.tile([128, 512], F32, name="abt")
            cct = p12.tile([128, 512], F32, name="cct")
            t1t = p12.tile([128, 512], F32, name="t1t")
            idx16 = p12.tile([128, T], I16, name="idx16")
            lists = p12.tile([128, S8MAX], I16, name="lists")
            lf = p12.tile([128, S8MAX], F32, name="lf")

            strip_dmas = []

            # ---------------- P1: router ----------------
            # Strip DMA order h0,h1,l0,h2,l1,...,h7,l6,l7 on the ACT queue;
            # per chunk the xh@gh + xh@gl terms run at hi-strip arrival and
            # the xl@gh term (plus the gating chain) one strip behind, so
            # PE/DVE work overlaps the strip loads.
            with tc.tile_pool(name="p1xh", bufs=2) as p1xh, \
                 tc.tile_pool(name="p1xl", bufs=2) as p1xl, \
                 tc.tile_pool(name="p1s", bufs=2) as p1s, \
                 tc.tile_pool(name="p1g", bufs=4) as p1g, \
                 tc.tile_pool(name="p1pl", bufs=3, space="PSUM") as p1pl, \
                 tc.tile_pool(name="p1p8", bufs=2, space="PSUM") as p1p8, \
                 tc.tile_pool(name="p2pb", bufs=2, space="PSUM") as p2pb, \
                 tc.tile_pool(name="p2pr", bufs=1, space="PSUM") as p2pr:

                CPS = SW // 128  # chunks per strip

                def load_strip(st):
                    xs = p1xh.tile([128, 2, HC, SW], BF, tag="xhl",
                                   name=f"xs{st}")
                    if st < 1:
                        # halved: faster first arrival (512B descriptors
                        # still run at full DMA bandwidth; smaller would
                        # halve it)
                        for q2 in range(2):
                            qs = slice(q2 * (SW // 2), (q2 + 1) * (SW // 2))
                            strip_dmas.append(nc.sync.dma_start(
                                xs[:, :, :, qs],
                                xthl_in.ap()[st][:, :, :, qs]))
                    else:
                        strip_dmas.append(
                            nc.sync.dma_start(xs[:], xthl_in.ap()[st]))
                    return xs

                def router_chunk(c, xs):
                    cc = c % CPS
                    csl = slice(cc * 128, (cc + 1) * 128)
                    lg = p1pl.tile([128, E], F32, tag="lg", name=f"lg{c}")
                    for hc in range(HC):
                        nc.tensor.matmul(lg[:], lhsT=xs[:, 0, hc, csl],
                                         rhs=gwh[:, hc, :],
                                         start=(hc == 0), stop=False)
                    for hc in range(HC):
                        nc.tensor.matmul(lg[:], lhsT=xs[:, 0, hc, csl],
                                         rhs=gwl[:, hc, :],
                                         start=False, stop=False)
                    for hc in range(HC):
                        nc.tensor.matmul(lg[:], lhsT=xs[:, 1, hc, csl],
                                         rhs=gwh[:, hc, :],
                                         start=False, stop=(hc == HC - 1))
                    return lg

                def gating_chunk(c, lg):
                    rows = slice(c * 128, (c + 1) * 128)
                    sc = p1s.tile([128, E], F32, tag="sc", name=f"sc{c}")
                    nc.scalar.activation(sc[:], lg[:], AF.Sigmoid)
                    sel = p1s.tile([128, E], F32, tag="sel", name=f"se{c}")
                    nc.vector.tensor_add(sel[:], sc[:], bias_s[:])
                    mx8 = p1s.tile([128, 8], F32, tag="mx8", name=f"mx{c}")
                    nc.vector.max(out=mx8[:], in_=sel[:])
                    msel = p1s.tile([128, E], F32, tag="msel", name=f"ms{c}")
                    nc.vector.match_replace(out=msel[:], in_to_replace=mx8[:],
                                            in_values=sel[:], imm_value=-1e30)
                    maskc = p1s.tile([128, E], F32, tag="maskc", name=f"mc{c}")
                    nc.vector.tensor_scalar(maskc[:], msel[:], -1e29, None,
                                            op0=ALU.is_le)
                    wm = p1s.tile([128, E], F32, tag="wm", name=f"wm{c}")
                    ssum = p1s.tile([128, 1], F32, tag="ssum", name=f"ss{c}")
                    nc.vector.scalar_tensor_tensor(out=wm[:], in0=sc[:],
                                                   scalar=0.0, in1=maskc[:],
                                                   op0=ALU.add, op1=ALU.mult,
                                                   accum_out=ssum[:])
                    winv = p1s.tile([128, 1], F32, tag="winv", name=f"wv{c}")
                    nc.vector.reciprocal(winv[:], ssum[:])
                    gt = p1g.tile([128, E], F32, tag="gt", name=f"gt{c}")
                    nc.vector.tensor_scalar_mul(gt[:], wm[:], winv[:])
                    # ACT queue: strips own SP; consts and weights queue
                    # behind these writes, which drain progressively.
                    gating_chunk.last_gats = nc.scalar.dma_start(
                        gatS_ap[rows, :], gt[:])
                    tp16 = p1p8.tile([128, 128], F32, tag="tp16")
                    nc.tensor.transpose(tp16[:GP, :], gt[:, 0:GP], identf[:])
                    gdst = gts[c // 4]
                    gcol0 = (c % 4) * 128
                    nc.scalar.activation(gdst[0:GP, gcol0:gcol0 + 128],
                                         tp16[:GP, :], AF.Copy)

                lgs = {}
                strips = {}
                for c in range(NCHUNK + 1):
                    if c < NCHUNK:
                        st = c // CPS
                        if c % CPS == 0:
                            strips[st] = load_strip(st)
                        lgs[c] = router_chunk(c, strips[st])
                    if c >= 1:
                        gating_chunk(c - 1, lgs.pop(c - 1))
                nc.sync.dma_start(gatS_ap[T:TP, :], zf[:])
                # slot-0 weights: explicitly ordered after the last strips
                # (the scheduler would otherwise hoist these dep-free DMAs
                # into the middle of the strip sequence)
                wg0d = nc.scalar.dma_start(wgus0[:], wgu_in.ap()[0])
                wd0d = nc.scalar.dma_start(wds0[:], wd_in.ap()[0])
                _add_dep_helper(wg0d.ins, gating_chunk.last_gats.ins, False,
                                "wgu0 after last gatS")

            # ---------------- P2: dispatch ----------------
                for _t, _src in ((dat16, dat_in), (e8, e8_in),
                                 (rev, rev_in), (rod, rod_in),
                                 (labs, labs_in), (lthr, lthr_in),
                                 (loff, loff_in)):
                    nc.scalar.dma_start(_t[:], _src.ap())

                # segmented mask/scan/window chain (512 tokens per segment):
                # runs concurrently with P1 as gTS columns land.
                csprev = None
                for sk in range(T // 512):
                        hf, nt = sk // 4, sk % 4
                        nc.vector.tensor_scalar(mbt[:], gts[sk][:], 0.0,
                                                None, op0=ALU.is_gt)
                        cs = csA if sk % 2 == 0 else csB
                        ini = 0.0 if csprev is None else csprev[:, 511:512]
                        nc.vector.tensor_tensor_scan(cs[:], data0=mbt[:],
                                                     data1=mbt[:],
                                                     initial=ini,
                                                     op0=ALU.add,
                                                     op1=ALU.bypass)
                        csprev = cs
                        nc.vector.tensor_mul(qht[:], cs[:], mbt[:])
                        # lane ch=8s+p: valid slot idx = (q-1)-start_s-S8_s*p
                        # iff in [0, S8_s): bp = q; ab = |bp + labs|;
                        # cc = ab <= lthr; idx16 = (bp + loff)*cc - 1.
                        bp = p2pb.tile([128, 512], F32, tag="bp")
                        nc.tensor.matmul(bp[:], lhsT=e8[:, :], rhs=qht[:],
                                         start=True, stop=True)
                        nc.scalar.activation(abt[:], bp[:], AF.Abs,
                                             bias=labs[:])
                        nc.vector.tensor_scalar(cct[:], abt[:], lthr[:],
                                                None, op0=ALU.is_le)
                        nc.vector.scalar_tensor_tensor(
                            out=t1t[:], in0=bp[:], scalar=loff[:], in1=cct[:],
                            op0=ALU.add, op1=ALU.mult)
                        col = hf * TH + nt * 512
                        nc.vector.tensor_scalar_add(idx16[:, col:col + 512],
                                                    t1t[:], -1.0)

                lsc = nc.gpsimd.local_scatter(out_ap=lists[:],
                                              data_ap=dat16[:],
                                              idxs_ap=idx16[:], channels=128,
                                              num_elems=S8MAX, num_idxs=T)
                ll2 = nc.gpsimd.load_library(library_config.mlp)
                _add_dep_helper(lsc.ins, ll1.ins, True,
                                "lib order: ls after load7")
                _add_dep_helper(ll2.ins, lsc.ins, True,
                                "lib order: load3 after ls")
                # wd0 transfer would jam the DMA pool right when the first
                # x-gathers need it; not needed until the first stage_D
                _add_dep_helper(wd0d.ins, lsc.ins, False, "wd0 after ls")

                nc.vector.tensor_copy(lf[:], lists[:])
                # re-wrap 8-lane channels into the 16-lane gather layout:
                # idxw[row, s, c] = lists[8s + (row%16)//2, c + S16_s*(row%2)]
                # (+T so empty slots (0) hit the zero-row sentinel)
                off = 0
                for s in range(G):
                    if profile[s] == 0:
                        continue
                    rp = p2pr.tile([128, 512], F32, tag="rp")
                    nc.tensor.matmul(rp[:, 0:S16[s]], lhsT=rev[:, s, :],
                                     rhs=lf[:, 0:S16[s]],
                                     start=True, stop=False)
                    nc.tensor.matmul(rp[:, 0:S16[s]], lhsT=rod[:, s, :],
                                     rhs=lf[:, S16[s]:S8[s]],
                                     start=False, stop=True)
                    nc.vector.tensor_scalar_add(idxw[:, off:off + S16[s]],
                                                rp[:, 0:S16[s]], float(T))
                    off += S16[s]

            p12_cm.__exit__(None, None, None)

            # ---------------- P3: expert SwiGLU GEMMs ----------------
            # software pipeline: per row-tile, stage A = g/u matmuls (two
            # halves), B = silu chain (ACT/DVE), C = PE transposes of h,
            # D = down matmuls + scale + scatter.  Emission order puts
            # C(i-1) between A's two halves and D(i-1) after A(i) so the
            # in-order PE queue never stalls on the B/C copies.
            swdge = []
            with tc.tile_pool(name="px", bufs=2) as px, \
                 tc.tile_pool(name="pgg", bufs=2) as pgg, \
                 tc.tile_pool(name="pa", bufs=3) as pa, \
                 tc.tile_pool(name="psG", bufs=4, space="PSUM") as psG, \
                 tc.tile_pool(name="psT", bufs=2, space="PSUM") as psT, \
                 tc.tile_pool(name="psY", bufs=2, space="PSUM") as psY:
                HW2 = II // 2  # 384

                tiles_list = []       # (slot, rt, xte, rsl, wgus, wds, ggat)
                off = 0
                for s in range(G):
                    TC = profile[s]
                    if TC == 0:
                        continue
                    iws = idxw[:, off:off + S16[s]]
                    off += S16[s]
                    if s == 0:
                        wgus, wds = wgus0, wds0
                    else:
                        wgus = pwg.tile([128, HC, 2 * II], BF, tag="wgu")
                        d1 = nc.scalar.dma_start(wgus[:], wgu_in.ap()[s])
                        wds = pwd.tile([128, IC, H], BF, tag="wd")
                        d2 = nc.scalar.dma_start(wds[:], wd_in.ap()[s])
                        if s == 1:
                            _add_dep_helper(d1.ins, lsc.ins, False,
                                            "wgu1 after ls")
                            _add_dep_helper(d2.ins, lsc.ins, False,
                                            "wd1 after ls")
                    ggat = pgg.tile([128, TC, E], F32, tag="gg")
                    for g0 in range(0, TC, 4):
                        gn = min(4, TC - g0)
                        rn = gn * 128
                        xte = px.tile([128, HC, rn], BF, tag="xt")
                        g2 = nc.gpsimd.dma_gather(
                            out_ap=xte[:], in_ap=xbf_ap[:],
                            idxs_ap=iws[:, g0 * 8:(g0 + gn) * 8],
                            num_idxs=rn, num_idxs_reg=rn, elem_size=H,
                            transpose=True)
                        swdge.append(g2)
                        if g0 == 0:
                            # gating gather after the first x-gather: it is
                            # only needed at stage D, keep it off the
                            # critical path to the first matmuls.
                            g1 = nc.gpsimd.dma_gather(
                                out_ap=ggat[:], in_ap=gatS_ap[:],
                                idxs_ap=iws,
                                num_idxs=TC * 128, num_idxs_reg=TC * 128,
                                elem_size=E)
                            swdge.append(g1)
                        for rti in range(gn):
                            rt = g0 + rti
                            rsl = slice(rti * 128, (rti + 1) * 128)
                            tiles_list.append((s, rt, xte, rsl, wgus, wds,
                                               ggat, iws))

                def stage_A(i, half2):
                    s, rt, xte, rsl, wgus, wds, ggat, iws = tiles_list[i]
                    io = half2 * HW2
                    gph = psG.tile([128, HW2], F32, tag="gu",
                                   name=f"gp{i}_{half2}")
                    uph = psG.tile([128, HW2], F32, tag="gu",
                                   name=f"up{i}_{half2}")
                    for hc in range(HC):
                        for ps, io2 in ((gph, io), (uph, II + io)):
                            nc.tensor.matmul(
                                ps[:], lhsT=xte[:, hc, rsl],
                                rhs=wgus[:, hc, io2:io2 + HW2],
                                start=(hc == 0), stop=(hc == HC - 1))
                    return gph, uph

                def stage_B(i, half2, gph, uph, hT):
                    gsh = pa.tile([128, HW2], F32, tag="gs",
                                  name=f"gs{i}_{half2}")
                    nc.scalar.activation(gsh[:], gph[:], AF.Sigmoid)
                    m1h = pa.tile([128, HW2], F32, tag="m1",
                                  name=f"m1{i}_{half2}")
                    nc.vector.tensor_mul(m1h[:], gsh[:], gph[:])
                    hbh = pa.tile([128, HW2], BF, tag="hbf",
                                  name=f"hb{i}_{half2}")
                    nc.vector.tensor_mul(hbh[:], m1h[:], uph[:])
                    return hbh

                def stage_C(i, half2, hbh, hT):
                    tp = psT.tile([128, 3, 128], BF, tag="tp")
                    for ici in range(IC // 2):
                        nc.tensor.transpose(
                            tp[:, ici, :],
                            hbh[:, ici * 128:(ici + 1) * 128],
                            identb[:])
                    i0 = half2 * (IC // 2)
                    if half2 == 0:
                        nc.vector.tensor_copy(hT[:, i0:i0 + 3, :], tp[:])
                    else:
                        nc.scalar.activation(hT[:, i0:i0 + 3, :], tp[:],
                                             AF.Copy)

                def stage_D(i, hT):
                    s, rt, xte, rsl, wgus, wds, ggat, iws = tiles_list[i]
                    ysc = pa.tile([128, 1, H], BF, tag="ysc", name=f"ys{i}")
                    gcol = ggat[:, rt, s:s + 1]
                    for n3 in range(3):
                        yp = psY.tile([128, 512], F32, tag="y")
                        for ic in range(IC):
                            nc.tensor.matmul(
                                yp[:], lhsT=hT[:, ic, :],
                                rhs=wds[:, ic, n3 * 512:(n3 + 1) * 512],
                                start=(ic == 0), stop=(ic == IC - 1))
                        nc.vector.tensor_scalar_mul(
                            ysc[:, 0, n3 * 512:(n3 + 1) * 512], yp[:], gcol)
                    s1 = nc.gpsimd.dma_scatter_add(
                        out_ap=pout_ap[:], in_ap=ysc[:],
                        idxs_ap=iws[:, rt * 8:rt * 8 + 8],
                        num_idxs=128, num_idxs_reg=128, elem_size=H)
                    swdge.append(s1)

                NT = len(tiles_list)
                state = {}   # i -> (hbh0, hbh1, hT)
                for i in range(NT + 1):
                    if i < NT:
                        hT = pa.tile([128, IC, 128], BF, tag="hT",
                                     name=f"hT{i}")
                        g0, u0 = stage_A(i, 0)
                        hb0 = stage_B(i, 0, g0, u0, hT)
                        if i >= 1:
                            hb0p, hb1p, hTp = state.pop(i - 1)
                            stage_C(i - 1, 0, hb0p, hTp)
                        g1_, u1_ = stage_A(i, 1)
                        hb1 = stage_B(i, 1, g1_, u1_, hT)
                        if i >= 1:
                            stage_C(i - 1, 1, hb1p, hTp)
                            stage_D(i - 1, hTp)
                        state[i] = (hb0, hb1, hT)
                    else:
                        hb0p, hb1p, hTp = state.pop(i - 1)
                        stage_C(i - 1, 0, hb0p, hTp)
                        stage_C(i - 1, 1, hb1p, hTp)
                        stage_D(i - 1, hTp)
            for ins in swdge:
                _add_dep_helper(ins.ins, ll2.ins, False,
                                "lib order: mlp ops after load3")

    nc.compile()
    return nc


_NC_CACHE = {}


def _get_program(profile):
    if profile not in _NC_CACHE:
        _NC_CACHE[profile] = _build_program(profile)
    return _NC_CACHE[profile]


# --------------------------------------------------------------------------
# host-side input prep
# --------------------------------------------------------------------------

def _split_bf16(a):
    hi = a.astype(BF16)
    lo = (a - hi.astype(np.float32)).astype(BF16)
    return hi, lo


def make_in_maps(hidden_states, gate_w, routing_bias, w_gate, w_up, w_down,
                 profile, bins):
    G = len(profile)
    S8 = [16 * tc for tc in profile]
    S16 = [8 * tc for tc in profile]

    x = np.asarray(hidden_states, dtype=np.float32)
    gw = np.asarray(gate_w, dtype=np.float32)
    rb = np.asarray(routing_bias, dtype=np.float32)
    wgt = np.asarray(w_gate)
    wut = np.asarray(w_up)
    wdt = np.asarray(w_down)

    xh, xl = _split_bf16(x)
    xbf = np.zeros((TP, H), dtype=BF16)
    xbf[:T] = xh

    def strips(a):
        # [T, H] -> [NSTRIP, 128, HC, SW];  [st, p, hc, t] = a[st*SW+t, hc*128+p]
        return np.ascontiguousarray(
            a.reshape(NSTRIP, SW, HC, 128).transpose(0, 3, 2, 1))

    xthl = np.ascontiguousarray(
        np.stack([strips(xh), strips(xl)], axis=2))

    gwh32 = gw.astype(BF16).astype(np.float32)
    gwl32 = (gw - gwh32).astype(BF16).astype(np.float32)

    identf = np.eye(128, dtype=np.float32)
    identb = np.eye(128).astype(BF16)
    dat16 = np.tile(np.arange(-T, 0, dtype=np.int16), (128, 1))
    e8 = np.zeros((16, 128), np.float32)
    for s in range(min(G, 16)):
        e8[s, 8 * s:8 * s + 8] = 1.0
    rev = np.zeros((128, G, 128), np.float32)
    rod = np.zeros((128, G, 128), np.float32)
    for s in range(G):
        for row in range(128):
            q = row % 16
            ch = 8 * s + q // 2
            if row % 2 == 0:
                rev[ch, s, row] = 1.0
            else:
                rod[ch, s, row] = 1.0

    def gwtr(a):
        # [E, H] (fp32) -> [128, HC, E] bf16
        return np.ascontiguousarray(
            a.T.reshape(HC, 128, E).transpose(1, 0, 2)).astype(BF16)

    in_maps = []
    for c in range(NCORES):
        pieces = bins[c]
        slot_exp = [p[0] for p in pieces]
        used = set(e for e in slot_exp if e >= 0)
        rest = [e for e in range(E) if e not in used]
        ri = 0
        perm = []
        for e in slot_exp:
            if e >= 0:
                perm.append(e)
            else:
                perm.append(rest[ri])
                ri += 1
        perm += rest[ri:]
        assert sorted(perm) == list(range(E))
        perm = np.array(perm)

        labs = np.zeros((128, 1), np.float32)
        lthr = np.full((128, 1), -1.0, np.float32)
        loff = np.zeros((128, 1), np.float32)
        for s in range(G):
            e, st_tile, ntiles = pieces[s]
            if e < 0 or ntiles == 0:
                continue
            start = st_tile * 128
            for p in range(8):
                ch = 8 * s + p
                base = start + S8[s] * p
                labs[ch] = -(base + 1) - (S8[s] - 1) / 2.0
                lthr[ch] = (S8[s] - 1) / 2.0
                loff[ch] = -base

        wgu = np.zeros((G, 128, HC, 2 * II), BF16)
        wd = np.zeros((G, 128, IC, H), BF16)
        for s in range(G):
            e = pieces[s][0]
            if e < 0 or profile[s] == 0:
                continue
            wg_t = wgt[e].T.reshape(HC, 128, II).transpose(1, 0, 2)
            wu_t = wut[e].T.reshape(HC, 128, II).transpose(1, 0, 2)
            wgu[s][:, :, :II] = wg_t.astype(BF16)
            wgu[s][:, :, II:] = wu_t.astype(BF16)
            wd[s] = wdt[e].T.reshape(IC, 128, H).transpose(1, 0, 2).astype(BF16)

        in_maps.append(dict(
            xthl=xthl, xbfh=xbf,
            gwh=gwtr(gwh32[perm]),
            gwl=gwtr(gwl32[perm]),
            biasb=np.tile(rb[perm][None, :], (128, 1)).astype(np.float32),
            identf=identf, identb=identb, dat16=dat16, e8=e8,
            rev=rev, rod=rod, labs=labs, lthr=lthr, loff=loff,
            wgu=wgu, wd=wd,
        ))
    return in_maps


def kernel(hidden_states, gate_w, routing_bias, w_gate, w_up, w_down,
           num_global_tokens=None, max_num_tokens_per_gpu=None, **_unused):
    x = np.asarray(hidden_states, np.float32)
    gw = np.asarray(gate_w, np.float32)
    rb = np.asarray(routing_bias, np.float32)
    loads = _host_loads(x, gw, rb)
    profile, bins = _schedule(loads)
    nc = _get_program(profile)
    in_maps = make_in_maps(x, gw, rb, w_gate, w_up, w_down, profile, bins)
    res = bass_utils.run_bass_kernel_spmd(nc, in_maps,
                                          core_ids=list(range(NCORES)))
    out = np.zeros((T, H), dtype=np.float32)
    for c in range(NCORES):
        out += np.asarray(res.results[c]["pout"])[:T].astype(np.float32)
    return out


# revision 33
# speedup vs baseline: 1.0507x; 1.0221x over previous
"""MiniMax-M2 sparse MoE block on 8 Trainium2 NeuronCores (expert-parallel).

v2: load-balanced adaptive schedule + split-precision router.

T=4096 tokens, H=1536, I=768, E=64 experts, top-8 sigmoid routing.

Key ideas vs v1 (8 static experts/core x 7 capacity tiles = 56 row-tiles):
  * PE matmul cost scales with the output free-dim only, so P3 cost =
    (#row-tiles) x ~11.8us.  The per-core minimum is
    ceil(sum_e ceil(load_e/128) / 8) ~= 38 tiles.  A host-side scheduler
    (the router is recomputed on host in fp32 purely to pick the schedule;
    all math still runs on device) bin-packs experts into 8 cores with an
    identical per-slot tile profile, splitting experts across cores when
    needed.
  * Split-precision router: x = xh + xl (bf16 each), gw = gh + gl;
    logits = xh@gh + xl@gh + xh@gl accumulated in fp32 PSUM.  Zero top-8
    flips vs fp32 (logit err ~1.6e-5) at bf16 matmul speed.  The
    transposed x operands are host-prepared, so P1 needs no PE transposes.
  * Dispatch (P2) generalized to per-slot (expert, start, ntiles) pieces;
    all per-slot binding constants are input data, only the slot
    tile-count profile is program structure (identical across cores ->
    SPMD-safe).  local_scatter uses 8-lane channels (16 slots max in one
    128-channel page); two matmuls per slot re-wrap the lists into the
    16-lane replicated layout the SWDGE gathers expect.
  * pout accumulates in bf16 (halves scatter-add traffic); host sums the
    8 partials in fp32.
"""

import math

import numpy as np
import ml_dtypes

import concourse.bass as bass
import concourse.mybir as mybir
import concourse.tile as tile
from concourse import bacc, library_config
from concourse import bass_utils
from concourse.bass import _add_dep_helper

BF16 = ml_dtypes.bfloat16

T = 4096
H = 1536
II = 768
E = 64
K = 8
NCORES = 8
PAD = 16          # safety margin on host-predicted expert loads
TP = T + 16       # padded token rows; rows T.. are the zero sentinel
AF = mybir.ActivationFunctionType
ALU = mybir.AluOpType
F32 = mybir.dt.float32
F32R = mybir.dt.float32r
BF = mybir.dt.bfloat16
I16 = mybir.dt.int16
HC = H // 128      # 12
IC = II // 128     # 6
NCHUNK = T // 128  # 32
TH = T // 2
GP = 16            # gatS width / max slots
NSTRIP = 8
SW = T // NSTRIP   # 512


# --------------------------------------------------------------------------
# host-side routing + scheduling
# --------------------------------------------------------------------------

def _host_loads(x, gw, rb):
    """fp32 router on host; per-expert token counts (schedule only)."""
    logits = x.astype(np.float32) @ gw.astype(np.float32).T
    scores = 1.0 / (1.0 + np.exp(-logits))
    sel = scores + rb.astype(np.float32)[None, :]
    kth = np.partition(sel, E - K, axis=1)[:, E - K][:, None]
    return (sel >= kth).sum(axis=0).astype(int)


def _schedule(loads):
    """Bin-pack experts into 8 cores x G slots.

    Returns (profile, bins): profile = per-slot tile counts (descending,
    shared across cores); bins[c] = list of (expert, start_tile, ntiles)
    sorted descending by ntiles, padded with (-1, 0, 0) to len(profile).
    No expert appears twice in one bin.
    """
    import random
    tiles = [max(1, math.ceil((l + PAD) / 128)) for l in loads]
    TT = sum(tiles)
    Q = math.ceil(TT / NCORES)

    def rankmax(bins):
        g = max(len(b) for b in bins)
        return sum(max((b[r][2] if r < len(b) else 0) for b in bins)
                   for r in range(g))

    def valid(b):
        return len({p[0] for p in b}) == len(b)

    def lpt():
        order = sorted(range(E), key=lambda e: -tiles[e])
        bins = [[] for _ in range(NCORES)]
        sums = [0] * NCORES
        for e in order:
            c = min(range(NCORES), key=lambda i: (sums[i], len(bins[i])))
            bins[c].append((e, 0, tiles[e]))
            sums[c] += tiles[e]
        for b in bins:
            b.sort(key=lambda p: -p[2])
        return bins

    def split_fill():
        order = sorted(range(E), key=lambda e: -tiles[e])
        bins = [[] for _ in range(NCORES)]
        c, rem = 0, Q
        for e in order:
            t, st = tiles[e], 0
            while t > 0:
                if rem == 0 and c < NCORES - 1:
                    c += 1
                    rem = Q
                take = min(t, rem)
                bins[c].append((e, st, take))
                st += take
                t -= take
                rem -= take
        for b in bins:
            b.sort(key=lambda p: -p[2])
        return bins

    rng = random.Random(0)

    def cost_of(bins):
        return rankmax(bins) * 100 + max(len(b) for b in bins)

    def local_search(base, iters=20000):
        bins = [list(b) for b in base]
        best = cost_of(bins)
        for _ in range(iters):
            c1, c2 = rng.randrange(NCORES), rng.randrange(NCORES)
            if c1 == c2 or not bins[c1]:
                continue
            i1 = rng.randrange(len(bins[c1]))
            saved = (list(bins[c1]), list(bins[c2]))
            r = rng.random()
            if r < 0.4 and bins[c2]:
                i2 = rng.randrange(len(bins[c2]))
                bins[c1][i1], bins[c2][i2] = bins[c2][i2], bins[c1][i1]
            elif r < 0.7:
                bins[c2].append(bins[c1].pop(i1))
            else:
                e, st, nt = bins[c1][i1]
                if nt < 2:
                    continue
                k = rng.randrange(1, nt)
                bins[c1][i1] = (e, st, nt - k)
                bins[c2].append((e, st + nt - k, k))
            if valid(bins[c1]) and valid(bins[c2]) and \
                    max(len(bins[c1]), len(bins[c2])) <= GP:
                for b in (bins[c1], bins[c2]):
                    b.sort(key=lambda p: -p[2])
                cost = cost_of(bins)
                if cost <= best:
                    best = cost
                    continue
            bins[c1], bins[c2] = saved
        return bins

    def profile_construct():
        """Enumerate slot profiles (desc partitions of S, parts<=7) in
        (S, nslots) order; rank-deal items into 8 bins with splitting;
        return bins for the first feasible profile."""
        maxpart = min(7, max(tiles))

        def partitions(S, maxp, maxlen):
            if S == 0:
                yield []
                return
            if maxlen == 0:
                return
            for p in range(min(maxp, S), 0, -1):
                for rest in partitions(S - p, p, maxlen - 1):
                    yield [p] + rest

        import bisect

        def try_profile(P):
            # pool: desc list of (ntiles, expert, start)
            pool = sorted(((t, e, 0) for e, t in enumerate(tiles)),
                          reverse=True)
            bins = [[] for _ in range(NCORES)]
            used = [set() for _ in range(NCORES)]
            for pr in P:
                row = []
                for _ in range(NCORES):
                    if not pool:
                        row.append(None)
                        continue
                    t, e, st = pool.pop(0)
                    take = min(t, pr)
                    row.append((e, st, take))
                    if t > take:
                        item = (t - take, e, st + take)
                        bisect.insort(pool, item)
                        pool.reverse()
                        pool.sort(reverse=True)
                # assign row pieces to bins, avoiding same-expert repeats
                order = sorted(range(NCORES),
                               key=lambda c: sum(p[2] for p in bins[c]))
                pieces = [p for p in row if p is not None]
                taken = [False] * NCORES
                ok = True
                for p in pieces:
                    placed = False
                    for c in order:
                        if not taken[c] and p[0] not in used[c]:
                            bins[c].append(p)
                            used[c].add(p[0])
                            taken[c] = True
                            placed = True
                            break
                    if not placed:
                        ok = False
                        break
                if not ok:
                    return None
            if pool:
                return None
            for b in bins:
                b.sort(key=lambda p: -p[2])
            return bins

        for S in range(Q, Q + 3):
            for nlen in range(math.ceil(S / maxpart), GP + 1):
                for P in partitions(S, maxpart, nlen):
                    if len(P) != nlen:
                        continue
                    bins = try_profile(P)
                    if bins is not None:
                        return bins
        return None

    cands = []
    pc = profile_construct()
    if pc is not None and all(valid(b) and len(b) <= GP for b in pc):
        cands.append(pc)
    for base in (lpt(), split_fill()):
        if all(valid(b) and len(b) <= GP for b in base):
            cands.append(base)
            cands.append(local_search(base))
    bins = min(cands, key=cost_of)
    G = max(len(b) for b in bins)
    profile = tuple(
        max((b[r][2] if r < len(b) else 0) for b in bins) for r in range(G))
    bins = [list(b) + [(-1, 0, 0)] * (G - len(b)) for b in bins]
    return profile, bins


# --------------------------------------------------------------------------
# device program (structure depends only on `profile`)
# --------------------------------------------------------------------------

def _build_program(profile):
    G = len(profile)
    S8 = [16 * tc for tc in profile]    # per-channel columns (8-lane wrap)
    S16 = [8 * tc for tc in profile]    # idxw columns (16-lane gather wrap)
    S8MAX = max(S8)
    SWT = sum(S16)

    nc = bacc.Bacc("TRN2", target_bir_lowering=False, debug=False,
                   enable_asserts=False)

    xthl_in = nc.dram_tensor("xthl", [NSTRIP, 128, 2, HC, SW], BF,
                             kind="ExternalInput")
    xbf_in = nc.dram_tensor("xbfh", [TP, H], BF, kind="ExternalInput")
    gwh_in = nc.dram_tensor("gwh", [128, HC, E], BF, kind="ExternalInput")
    gwl_in = nc.dram_tensor("gwl", [128, HC, E], BF, kind="ExternalInput")
    bias_in = nc.dram_tensor("biasb", [128, E], F32, kind="ExternalInput")
    idf_in = nc.dram_tensor("identf", [128, 128], F32, kind="ExternalInput")
    idb_in = nc.dram_tensor("identb", [128, 128], BF, kind="ExternalInput")
    dat_in = nc.dram_tensor("dat16", [128, T], I16, kind="ExternalInput")
    e8_in = nc.dram_tensor("e8", [16, 128], F32R, kind="ExternalInput")
    rev_in = nc.dram_tensor("rev", [128, G, 128], F32, kind="ExternalInput")
    rod_in = nc.dram_tensor("rod", [128, G, 128], F32, kind="ExternalInput")
    labs_in = nc.dram_tensor("labs", [128, 1], F32, kind="ExternalInput")
    lthr_in = nc.dram_tensor("lthr", [128, 1], F32, kind="ExternalInput")
    loff_in = nc.dram_tensor("loff", [128, 1], F32, kind="ExternalInput")
    wgu_in = nc.dram_tensor("wgu", [G, 128, HC, 2 * II], BF, kind="ExternalInput")
    wd_in = nc.dram_tensor("wd", [G, 128, IC, H], BF, kind="ExternalInput")

    gatS = nc.dram_tensor("gatS", [TP, E], F32, kind="Internal")
    pout = nc.dram_tensor("pout", [TP, H], BF, kind="ExternalOutput")

    xbf_ap = xbf_in.ap()
    gatS_ap = gatS.ap()
    pout_ap = pout.ap()

    with tile.TileContext(nc) as tc:
        with tc.tile_pool(name="const", bufs=1) as cp, \
             tc.tile_pool(name="pwg", bufs=2) as pwg, \
             tc.tile_pool(name="pwd", bufs=1) as pwd:
            identf = cp.tile([128, 128], F32)
            nc.scalar.dma_start(identf[:], idf_in.ap())
            identb = cp.tile([128, 128], BF)
            nc.scalar.dma_start(identb[:], idb_in.ap())
            gwh = cp.tile([128, HC, E], BF)
            nc.scalar.dma_start(gwh[:], gwh_in.ap())
            gwl = cp.tile([128, HC, E], BF)
            nc.scalar.dma_start(gwl[:], gwl_in.ap())
            bias_s = cp.tile([128, E], F32)
            nc.scalar.dma_start(bias_s[:], bias_in.ap())
            idxw = cp.tile([128, SWT], I16)
            # slot-0 weights from the persistent pools, DMA'd behind the
            # strips at the end of P1 (no SBUF overlap with P1/P2 pools ->
            # no WAR delay)
            wgus0 = pwg.tile([128, HC, 2 * II], BF, tag="wgu", name="wgup0")
            wds0 = pwd.tile([128, IC, H], BF, tag="wd", name="wdp0")
            ll1 = nc.gpsimd.load_library(library_config.local_scatter)

          # (indentation shim)
          # P1/P2-scoped constants
            p12_cm = tc.tile_pool(name="p12", bufs=1)
            p12 = p12_cm.__enter__()
            dat16 = p12.tile([128, T], I16)
            e8 = p12.tile([16, 128], F32R)
            rev = p12.tile([128, G, 128], F32)
            rod = p12.tile([128, G, 128], F32)
            labs = p12.tile([128, 1], F32)
            lthr = p12.tile([128, 1], F32)
            loff = p12.tile([128, 1], F32)
            zf = p12.tile([16, E], F32)
            nc.vector.memset(zf[:], 0.0)
            gts = [p12.tile([16, 512], BF, name=f"gts{k}")
                   for k in range(T // 512)]
            # P2 chain tiles, allocated early so they never overlap the P1
            # pools' SBUF (overlap would add a WAR wait for all of P1)
            mbt = p12.tile([16, 512], F32, name="mbt")
            csA = p12.tile([16, 512], F32, name="csA")
            csB = p12.tile([16, 512], F32, name="csB")
            qht = p12.tile([16, 512], F32R, name="qht")
            abt = p12.tile([128, 512], F32, name="abt")
            cct = p12.tile([128, 512], F32, name="cct")
            t1t = p12.tile([128, 512], F32, name="t1t")
            idx16 = p12.tile([128, T], I16, name="idx16")
            lists = p12.tile([128, S8MAX], I16, name="lists")
            lf = p12.tile([128, S8MAX], F32, name="lf")

            strip_dmas = []

            # ---------------- P1: router ----------------
            # Strip DMA order h0,h1,l0,h2,l1,...,h7,l6,l7 on the ACT queue;
            # per chunk the xh@gh + xh@gl terms run at hi-strip arrival and
            # the xl@gh term (plus the gating chain) one strip behind, so
            # PE/DVE work overlaps the strip loads.
            with tc.tile_pool(name="p1xh", bufs=2) as p1xh, \
                 tc.tile_pool(name="p1xl", bufs=2) as p1xl, \
                 tc.tile_pool(name="p1s", bufs=2) as p1s, \
                 tc.tile_pool(name="p1g", bufs=4) as p1g, \
                 tc.tile_pool(name="p1pl", bufs=3, space="PSUM") as p1pl, \
                 tc.tile_pool(name="p1p8", bufs=2, space="PSUM") as p1p8, \
                 tc.tile_pool(name="p2pb", bufs=2, space="PSUM") as p2pb, \
                 tc.tile_pool(name="p2pr", bufs=1, space="PSUM") as p2pr:

                CPS = SW // 128  # chunks per strip

                def load_strip(st):
                    xs = p1xh.tile([128, 2, HC, SW], BF, tag="xhl",
                                   name=f"xs{st}")
                    if st < 1:
                        # halved: faster first arrival (512B descriptors
                        # still run at full DMA bandwidth; smaller would
                        # halve it)
                        for q2 in range(2):
                            qs = slice(q2 * (SW // 2), (q2 + 1) * (SW // 2))
                            strip_dmas.append(nc.sync.dma_start(
                                xs[:, :, :, qs],
                                xthl_in.ap()[st][:, :, :, qs]))
                    else:
                        strip_dmas.append(
                            nc.sync.dma_start(xs[:], xthl_in.ap()[st]))
                    return xs

                def router_chunk(c, xs):
                    cc = c % CPS
                    csl = slice(cc * 128, (cc + 1) * 128)
                    lg = p1pl.tile([128, E], F32, tag="lg", name=f"lg{c}")
                    for hc in range(HC):
                        nc.tensor.matmul(lg[:], lhsT=xs[:, 0, hc, csl],
                                         rhs=gwh[:, hc, :],
                                         start=(hc == 0), stop=False)
                    for hc in range(HC):
                        nc.tensor.matmul(lg[:], lhsT=xs[:, 0, hc, csl],
                                         rhs=gwl[:, hc, :],
                                         start=False, stop=False)
                    for hc in range(HC):
                        nc.tensor.matmul(lg[:], lhsT=xs[:, 1, hc, csl],
                                         rhs=gwh[:, hc, :],
                                         start=False, stop=(hc == HC - 1))
                    return lg

                def gating_chunk(c, lg):
                    rows = slice(c * 128, (c + 1) * 128)
                    sc = p1s.tile([128, E], F32, tag="sc", name=f"sc{c}")
                    nc.scalar.activation(sc[:], lg[:], AF.Sigmoid)
                    sel = p1s.tile([128, E], F32, tag="sel", name=f"se{c}")
                    nc.vector.tensor_add(sel[:], sc[:], bias_s[:])
                    mx8 = p1s.tile([128, 8], F32, tag="mx8", name=f"mx{c}")
                    nc.vector.max(out=mx8[:], in_=sel[:])
                    msel = p1s.tile([128, E], F32, tag="msel", name=f"ms{c}")
                    nc.vector.match_replace(out=msel[:], in_to_replace=mx8[:],
                                            in_values=sel[:], imm_value=-1e30)
                    maskc = p1s.tile([128, E], F32, tag="maskc", name=f"mc{c}")
                    nc.vector.tensor_scalar(maskc[:], msel[:], -1e29, None,
                                            op0=ALU.is_le)
                    wm = p1s.tile([128, E], F32, tag="wm", name=f"wm{c}")
                    ssum = p1s.tile([128, 1], F32, tag="ssum", name=f"ss{c}")
                    nc.vector.scalar_tensor_tensor(out=wm[:], in0=sc[:],
                                                   scalar=0.0, in1=maskc[:],
                                                   op0=ALU.add, op1=ALU.mult,
                                                   accum_out=ssum[:])
                    winv = p1s.tile([128, 1], F32, tag="winv", name=f"wv{c}")
                    nc.vector.reciprocal(winv[:], ssum[:])
                    if c % 4 == 0:
                        gating_chunk.gtq = p1g.tile([128, 4, E], F32,
                                                    tag="gtq",
                                                    name=f"gtq{c // 4}")
                    gt = gating_chunk.gtq[:, c % 4, :]
                    nc.vector.tensor_scalar_mul(gt[:], wm[:], winv[:])
                    if c % 4 == 3:
                        # batched gating-table write on the Pool queue
                        # (strips own SP; ACT must stay clear for wgu0)
                        rows4 = slice((c - 3) * 128, (c + 1) * 128)
                        gating_chunk.last_gats = nc.gpsimd.dma_start(
                            gatS_ap[rows4, :].rearrange("(q p) e -> p q e",
                                                        p=128),
                            gating_chunk.gtq[:])
                    tp16 = p1p8.tile([128, 128], F32, tag="tp16")
                    nc.tensor.transpose(tp16[:GP, :], gt[:, 0:GP], identf[:])
                    gdst = gts[c // 4]
                    gcol0 = (c % 4) * 128
                    nc.scalar.activation(gdst[0:GP, gcol0:gcol0 + 128],
                                         tp16[:GP, :], AF.Copy)

                lgs = {}
                strips = {}
                for c in range(NCHUNK + 1):
                    if c < NCHUNK:
                        st = c // CPS
                        if c % CPS == 0:
                            strips[st] = load_strip(st)
                        lgs[c] = router_chunk(c, strips[st])
                    if c >= 1:
                        gating_chunk(c - 1, lgs.pop(c - 1))
                nc.sync.dma_start(gatS_ap[T:TP, :], zf[:])
                # slot-0 weights: explicitly ordered after the last strips
                # (the scheduler would otherwise hoist these dep-free DMAs
                # into the middle of the strip sequence)
                wg0d = nc.scalar.dma_start(wgus0[:], wgu_in.ap()[0])
                wd0d = nc.scalar.dma_start(wds0[:], wd_in.ap()[0])
                _add_dep_helper(wg0d.ins, strip_dmas[-1].ins, False,
                                "wgu0 after strips")

            # ---------------- P2: dispatch ----------------
                for _t, _src in ((dat16, dat_in), (e8, e8_in),
                                 (rev, rev_in), (rod, rod_in),
                                 (labs, labs_in), (lthr, lthr_in),
                                 (loff, loff_in)):
                    nc.scalar.dma_start(_t[:], _src.ap())

                # segmented mask/scan/window chain (512 tokens per segment):
                # runs concurrently with P1 as gTS columns land.
                csprev = None
                for sk in range(T // 512):
                        hf, nt = sk // 4, sk % 4
                        nc.vector.tensor_scalar(mbt[:], gts[sk][:], 0.0,
                                                None, op0=ALU.is_gt)
                        cs = csA if sk % 2 == 0 else csB
                        ini = 0.0 if csprev is None else csprev[:, 511:512]
                        nc.vector.tensor_tensor_scan(cs[:], data0=mbt[:],
                                                     data1=mbt[:],
                                                     initial=ini,
                                                     op0=ALU.add,
                                                     op1=ALU.bypass)
                        csprev = cs
                        nc.vector.tensor_mul(qht[:], cs[:], mbt[:])
                        # lane ch=8s+p: valid slot idx = (q-1)-start_s-S8_s*p
                        # iff in [0, S8_s): bp = q; ab = |bp + labs|;
                        # cc = ab <= lthr; idx16 = (bp + loff)*cc - 1.
                        bp = p2pb.tile([128, 512], F32, tag="bp")
                        nc.tensor.matmul(bp[:], lhsT=e8[:, :], rhs=qht[:],
                                         start=True, stop=True)
                        nc.scalar.activation(abt[:], bp[:], AF.Abs,
                                             bias=labs[:])
                        nc.vector.tensor_scalar(cct[:], abt[:], lthr[:],
                                                None, op0=ALU.is_le)
                        nc.vector.scalar_tensor_tensor(
                            out=t1t[:], in0=bp[:], scalar=loff[:], in1=cct[:],
                            op0=ALU.add, op1=ALU.mult)
                        col = hf * TH + nt * 512
                        nc.vector.tensor_scalar_add(idx16[:, col:col + 512],
                                                    t1t[:], -1.0)

                lsc = nc.gpsimd.local_scatter(out_ap=lists[:],
                                              data_ap=dat16[:],
                                              idxs_ap=idx16[:], channels=128,
                                              num_elems=S8MAX, num_idxs=T)
                ll2 = nc.gpsimd.load_library(library_config.mlp)
                _add_dep_helper(lsc.ins, ll1.ins, True,
                                "lib order: ls after load7")
                _add_dep_helper(ll2.ins, lsc.ins, True,
                                "lib order: load3 after ls")


                nc.vector.tensor_copy(lf[:], lists[:])
                # re-wrap 8-lane channels into the 16-lane gather layout:
                # idxw[row, s, c] = lists[8s + (row%16)//2, c + S16_s*(row%2)]
                # (+T so empty slots (0) hit the zero-row sentinel)
                off = 0
                for s in range(G):
                    if profile[s] == 0:
                        continue
                    rp = p2pr.tile([128, 512], F32, tag="rp")
                    nc.tensor.matmul(rp[:, 0:S16[s]], lhsT=rev[:, s, :],
                                     rhs=lf[:, 0:S16[s]],
                                     start=True, stop=False)
                    nc.tensor.matmul(rp[:, 0:S16[s]], lhsT=rod[:, s, :],
                                     rhs=lf[:, S16[s]:S8[s]],
                                     start=False, stop=True)
                    nc.vector.tensor_scalar_add(idxw[:, off:off + S16[s]],
                                                rp[:, 0:S16[s]], float(T))
                    off += S16[s]

            p12_cm.__exit__(None, None, None)

            P3_SLOT0_GGAT = [None]

            # ---------------- P3: expert SwiGLU GEMMs ----------------
            # software pipeline: per row-tile, stage A = g/u matmuls (two
            # halves), B = silu chain (ACT/DVE), C = PE transposes of h,
            # D = down matmuls + scale + scatter.  Emission order puts
            # C(i-1) between A's two halves and D(i-1) after A(i) so the
            # in-order PE queue never stalls on the B/C copies.
            swdge = []
            with tc.tile_pool(name="px", bufs=2) as px, \
                 tc.tile_pool(name="pgg", bufs=2) as pgg, \
                 tc.tile_pool(name="pa", bufs=3) as pa, \
                 tc.tile_pool(name="psG", bufs=4, space="PSUM") as psG, \
                 tc.tile_pool(name="psT", bufs=2, space="PSUM") as psT, \
                 tc.tile_pool(name="psY", bufs=2, space="PSUM") as psY:
                HW2 = II // 2  # 384

                tiles_list = []       # (slot, rt, xte, rsl, wgus, wds, ggat)
                off = 0
                for s in range(G):
                    TC = profile[s]
                    if TC == 0:
                        continue
                    iws = idxw[:, off:off + S16[s]]
                    off += S16[s]
                    if s == 0:
                        wgus, wds = wgus0, wds0
                    else:
                        wgus = pwg.tile([128, HC, 2 * II], BF, tag="wgu")
                        d1 = nc.scalar.dma_start(wgus[:], wgu_in.ap()[s])
                        wds = pwd.tile([128, IC, H], BF, tag="wd")
                        d2 = nc.scalar.dma_start(wds[:], wd_in.ap()[s])
                        if s == 1:
                            _add_dep_helper(d1.ins, P3_SLOT0_GGAT[0].ins,
                                            False, "wgu1 after slot0 ggat")
                            _add_dep_helper(d2.ins, P3_SLOT0_GGAT[0].ins,
                                            False, "wd1 after slot0 ggat")
                    ggat = pgg.tile([128, TC, E], F32, tag="gg")
                    for g0 in range(0, TC, 4):
                        gn = min(4, TC - g0)
                        rn = gn * 128
                        xte = px.tile([128, HC, rn], BF, tag="xt")
                        g2 = nc.gpsimd.dma_gather(
                            out_ap=xte[:], in_ap=xbf_ap[:],
                            idxs_ap=iws[:, g0 * 8:(g0 + gn) * 8],
                            num_idxs=rn, num_idxs_reg=rn, elem_size=H,
                            transpose=True)
                        swdge.append(g2)
                        if g0 == 0:
                            if s == 0:
                                # keep the big weight transfers out of the
                                # DMA pool until the critical first gathers
                                # have their slots
                                _add_dep_helper(wd0d.ins, g2.ins, False,
                                                "wd0 after first xte")
                            # gating gather after the first x-gather: it is
                            # only needed at stage D, keep it off the
                            # critical path to the first matmuls.
                            g1 = nc.gpsimd.dma_gather(
                                out_ap=ggat[:], in_ap=gatS_ap[:],
                                idxs_ap=iws,
                                num_idxs=TC * 128, num_idxs_reg=TC * 128,
                                elem_size=E)
                            swdge.append(g1)
                            if s == 0:
                                P3_SLOT0_GGAT[0] = g1
                        for rti in range(gn):
                            rt = g0 + rti
                            rsl = slice(rti * 128, (rti + 1) * 128)
                            tiles_list.append((s, rt, xte, rsl, wgus, wds,
                                               ggat, iws))

                def stage_A(i, half2):
                    s, rt, xte, rsl, wgus, wds, ggat, iws = tiles_list[i]
                    io = half2 * HW2
                    gph = psG.tile([128, HW2], F32, tag="gu",
                                   name=f"gp{i}_{half2}")
                    uph = psG.tile([128, HW2], F32, tag="gu",
                                   name=f"up{i}_{half2}")
                    for hc in range(HC):
                        for ps, io2 in ((gph, io), (uph, II + io)):
                            nc.tensor.matmul(
                                ps[:], lhsT=xte[:, hc, rsl],
                                rhs=wgus[:, hc, io2:io2 + HW2],
                                start=(hc == 0), stop=(hc == HC - 1))
                    return gph, uph

                def stage_B(i, half2, gph, uph, hT):
                    gsh = pa.tile([128, HW2], F32, tag="gs",
                                  name=f"gs{i}_{half2}")
                    nc.scalar.activation(gsh[:], gph[:], AF.Sigmoid)
                    m1h = pa.tile([128, HW2], F32, tag="m1",
                                  name=f"m1{i}_{half2}")
                    nc.vector.tensor_mul(m1h[:], gsh[:], gph[:])
                    hbh = pa.tile([128, HW2], BF, tag="hbf",
                                  name=f"hb{i}_{half2}")
                    nc.vector.tensor_mul(hbh[:], m1h[:], uph[:])
                    return hbh

                def stage_C(i, half2, hbh, hT):
                    tp = psT.tile([128, 3, 128], BF, tag="tp")
                    for ici in range(IC // 2):
                        nc.tensor.transpose(
                            tp[:, ici, :],
                            hbh[:, ici * 128:(ici + 1) * 128],
                            identb[:])
                    i0 = half2 * (IC // 2)
                    if half2 == 0:
                        nc.vector.tensor_copy(hT[:, i0:i0 + 3, :], tp[:])
                    else:
                        nc.scalar.activation(hT[:, i0:i0 + 3, :], tp[:],
                                             AF.Copy)

                def stage_D(i, hT):
                    s, rt, xte, rsl, wgus, wds, ggat, iws = tiles_list[i]
                    ysc = pa.tile([128, 1, H], BF, tag="ysc", name=f"ys{i}")
                    gcol = ggat[:, rt, s:s + 1]
                    for n3 in range(3):
                        yp = psY.tile([128, 512], F32, tag="y")
                        for ic in range(IC):
                            nc.tensor.matmul(
                                yp[:], lhsT=hT[:, ic, :],
                                rhs=wds[:, ic, n3 * 512:(n3 + 1) * 512],
                                start=(ic == 0), stop=(ic == IC - 1))
                        nc.vector.tensor_scalar_mul(
                            ysc[:, 0, n3 * 512:(n3 + 1) * 512], yp[:], gcol)
                    s1 = nc.gpsimd.dma_scatter_add(
                        out_ap=pout_ap[:], in_ap=ysc[:],
                        idxs_ap=iws[:, rt * 8:rt * 8 + 8],
                        num_idxs=128, num_idxs_reg=128, elem_size=H)
                    swdge.append(s1)

                NT = len(tiles_list)
                state = {}   # i -> (hbh0, hbh1, hT)
                for i in range(NT + 1):
                    if i < NT:
                        hT = pa.tile([128, IC, 128], BF, tag="hT",
                                     name=f"hT{i}")
                        g0, u0 = stage_A(i, 0)
                        hb0 = stage_B(i, 0, g0, u0, hT)
                        if i >= 1:
                            hb0p, hb1p, hTp = state.pop(i - 1)
                            stage_C(i - 1, 0, hb0p, hTp)
                        g1_, u1_ = stage_A(i, 1)
                        hb1 = stage_B(i, 1, g1_, u1_, hT)
                        if i >= 1:
                            stage_C(i - 1, 1, hb1p, hTp)
                            stage_D(i - 1, hTp)
                        state[i] = (hb0, hb1, hT)
                    else:
                        hb0p, hb1p, hTp = state.pop(i - 1)
                        stage_C(i - 1, 0, hb0p, hTp)
                        stage_C(i - 1, 1, hb1p, hTp)
                        stage_D(i - 1, hTp)
            for ins in swdge:
                _add_dep_helper(ins.ins, ll2.ins, False,
                                "lib order: mlp ops after load3")

    nc.compile()
    return nc


_NC_CACHE = {}


def _get_program(profile):
    if profile not in _NC_CACHE:
        _NC_CACHE[profile] = _build_program(profile)
    return _NC_CACHE[profile]


# --------------------------------------------------------------------------
# host-side input prep
# --------------------------------------------------------------------------

def _split_bf16(a):
    hi = a.astype(BF16)
    lo = (a - hi.astype(np.float32)).astype(BF16)
    return hi, lo


def make_in_maps(hidden_states, gate_w, routing_bias, w_gate, w_up, w_down,
                 profile, bins):
    G = len(profile)
    S8 = [16 * tc for tc in profile]
    S16 = [8 * tc for tc in profile]

    x = np.asarray(hidden_states, dtype=np.float32)
    gw = np.asarray(gate_w, dtype=np.float32)
    rb = np.asarray(routing_bias, dtype=np.float32)
    wgt = np.asarray(w_gate)
    wut = np.asarray(w_up)
    wdt = np.asarray(w_down)

    xh, xl = _split_bf16(x)
    xbf = np.zeros((TP, H), dtype=BF16)
    xbf[:T] = xh

    def strips(a):
        # [T, H] -> [NSTRIP, 128, HC, SW];  [st, p, hc, t] = a[st*SW+t, hc*128+p]
        return np.ascontiguousarray(
            a.reshape(NSTRIP, SW, HC, 128).transpose(0, 3, 2, 1))

    xthl = np.ascontiguousarray(
        np.stack([strips(xh), strips(xl)], axis=2))

    gwh32 = gw.astype(BF16).astype(np.float32)
    gwl32 = (gw - gwh32).astype(BF16).astype(np.float32)

    identf = np.eye(128, dtype=np.float32)
    identb = np.eye(128).astype(BF16)
    dat16 = np.tile(np.arange(-T, 0, dtype=np.int16), (128, 1))
    e8 = np.zeros((16, 128), np.float32)
    for s in range(min(G, 16)):
        e8[s, 8 * s:8 * s + 8] = 1.0
    rev = np.zeros((128, G, 128), np.float32)
    rod = np.zeros((128, G, 128), np.float32)
    for s in range(G):
        for row in range(128):
            q = row % 16
            ch = 8 * s + q // 2
            if row % 2 == 0:
                rev[ch, s, row] = 1.0
            else:
                rod[ch, s, row] = 1.0

    def gwtr(a):
        # [E, H] (fp32) -> [128, HC, E] bf16
        return np.ascontiguousarray(
            a.T.reshape(HC, 128, E).transpose(1, 0, 2)).astype(BF16)

    in_maps = []
    for c in range(NCORES):
        pieces = bins[c]
        slot_exp = [p[0] for p in pieces]
        used = set(e for e in slot_exp if e >= 0)
        rest = [e for e in range(E) if e not in used]
        ri = 0
        perm = []
        for e in slot_exp:
            if e >= 0:
                perm.append(e)
            else:
                perm.append(rest[ri])
                ri += 1
        perm += rest[ri:]
        assert sorted(perm) == list(range(E))
        perm = np.array(perm)

        labs = np.zeros((128, 1), np.float32)
        lthr = np.full((128, 1), -1.0, np.float32)
        loff = np.zeros((128, 1), np.float32)
        for s in range(G):
            e, st_tile, ntiles = pieces[s]
            if e < 0 or ntiles == 0:
                continue
            start = st_tile * 128
            for p in range(8):
                ch = 8 * s + p
                base = start + S8[s] * p
                labs[ch] = -(base + 1) - (S8[s] - 1) / 2.0
                lthr[ch] = (S8[s] - 1) / 2.0
                loff[ch] = -base

        wgu = np.zeros((G, 128, HC, 2 * II), BF16)
        wd = np.zeros((G, 128, IC, H), BF16)
        for s in range(G):
            e = pieces[s][0]
            if e < 0 or profile[s] == 0:
                continue
            wg_t = wgt[e].T.reshape(HC, 128, II).transpose(1, 0, 2)
            wu_t = wut[e].T.reshape(HC, 128, II).transpose(1, 0, 2)
            wgu[s][:, :, :II] = wg_t.astype(BF16)
            wgu[s][:, :, II:] = wu_t.astype(BF16)
            wd[s] = wdt[e].T.reshape(IC, 128, H).transpose(1, 0, 2).astype(BF16)

        in_maps.append(dict(
            xthl=xthl, xbfh=xbf,
            gwh=gwtr(gwh32[perm]),
            gwl=gwtr(gwl32[perm]),
            biasb=np.tile(rb[perm][None, :], (128, 1)).astype(np.float32),
            identf=identf, identb=identb, dat16=dat16, e8=e8,
            rev=rev, rod=rod, labs=labs, lthr=lthr, loff=loff,
            wgu=wgu, wd=wd,
        ))
    return in_maps


def kernel(hidden_states, gate_w, routing_bias, w_gate, w_up, w_down,
           num_global_tokens=None, max_num_tokens_per_gpu=None, **_unused):
    x = np.asarray(hidden_states, np.float32)
    gw = np.asarray(gate_w, np.float32)
    rb = np.asarray(routing_bias, np.float32)
    loads = _host_loads(x, gw, rb)
    profile, bins = _schedule(loads)
    nc = _get_program(profile)
    in_maps = make_in_maps(x, gw, rb, w_gate, w_up, w_down, profile, bins)
    res = bass_utils.run_bass_kernel_spmd(nc, in_maps,
                                          core_ids=list(range(NCORES)))
    out = np.zeros((T, H), dtype=np.float32)
    for c in range(NCORES):
        out += np.asarray(res.results[c]["pout"])[:T].astype(np.float32)
    return out
